# revision 33
# baseline (speedup 1.0000x reference)
"""Bass/Trainium2 kernel for causal multi-head attention.

B=2, S=2048, DIM=2048, H=16 heads, HD=128.
Sharding: 8 cores = (batch b in 0..1) x (head-group g in 0..3).
Each core column-shards wq/wk/wv (4 heads = 512 cols), row-shards wo,
and produces a partial [S, DIM] output; the host sums the 4 partials
per batch (unshard of the row-sharded wo matmul).

Device-side layout trick: the host feeds q/k/v pre-transposed (qT =
q[b].T etc.), so every matmul contraction lands on the partition dim
with zero on-device transposes:
  xqT[hd, s] = wq_chunk.T @ qT_chunk          (proj, transposed out)
  xkT[hd, s] likewise; xv[s, hd] from vT as lhsT
  ST[k, q]   = xkT_chunk.T @ xqT_slice        (scores, transposed)
  softmax over k = partition dim: sum via ones-column matmul
  attnT[hd, q] = xv_chunk.T @ probsT          (PV; output pre-transposed)
  out[s, dout] = attnT_chunk.T @ wo_chunk     (final projection)
All matmul operands are float32r (TF32-like, full PE rate at N=512).
"""

import sys

sys.path.insert(0, "/opt/trn_rl_repo")

import numpy as np

B, S, DIM, H = 2, 2048, 2048, 16
HD = 128
NCORES = 8
GROUPS = 4  # head-groups (tensor parallel)
HPG = H // GROUPS  # 4 heads per group
DG = HPG * HD  # 512 = per-group projection width
P = 128
DC = DIM // P  # 16 din chunks
ST_N = S // 512  # 4 s-tiles of 512
SC_N = S // P  # 16 s-chunks of 128
SCALE = 1.0 / np.sqrt(HD)

_cache = {}


def _build(reps=1):
    from contextlib import ExitStack

    import concourse.bacc as bacc
    import concourse.mybir as mybir
    import concourse.tile as tile

    f32 = mybir.dt.float32
    f16 = mybir.dt.float16
    Exp = mybir.ActivationFunctionType.Exp

    nc = bacc.Bacc("TRN2", target_bir_lowering=False, debug=False,
                   num_devices=NCORES)

    # host-tiled layouts: every DMA reads a fully contiguous block
    qT = nc.dram_tensor("qT", [DC, ST_N, P, 512], f16, kind="ExternalInput")
    kT = nc.dram_tensor("kT", [DC, ST_N, P, 512], f16, kind="ExternalInput")
    vT = nc.dram_tensor("vT", [DC, ST_N, P, 512], f16, kind="ExternalInput")
    wq = nc.dram_tensor("wq", [4, 4, P, DG], f16, kind="ExternalInput")
    wk = nc.dram_tensor("wk", [4, 4, P, DG], f16, kind="ExternalInput")
    wv = nc.dram_tensor("wv", [4, 4, P, DG], f16, kind="ExternalInput")
    wo = nc.dram_tensor("wo", [4, P, DIM], f16, kind="ExternalInput")
    msk = nc.dram_tensor("msk", [P, 512], f16, kind="ExternalInput")
    onesd = nc.dram_tensor("onesd", [P, P], f16, kind="ExternalInput")
    neyed = nc.dram_tensor("neyed", [P, P], f16, kind="ExternalInput")
    bmaskd = nc.dram_tensor("bmaskd", [P, P], f16, kind="ExternalInput")
    outp = nc.dram_tensor("outp", [S, DIM], f16, kind="ExternalOutput")

    with tile.TileContext(nc) as tc, ExitStack() as ctx:
        # Resident for the whole kernel: projected activations + consts.
        resid = ctx.enter_context(tc.tile_pool(name="resid", bufs=1))
        _ = reps  # body below may be repeated for timing builds
        xqT = resid.tile([P, HPG, S], f16, tag="xqT")  # [hd, head, s]
        xkT = resid.tile([P, HPG, S], f16, tag="xkT")
        xv = resid.tile([P, SC_N, DG], f16, tag="xv")  # [s%128, s//128, dout]
        ones = resid.tile([P, P], f16, tag="ones")
        attnT = resid.tile([P, HPG, S], f16, tag="attnT")
        # single lower-triangle mask: tri[p, j] = (j >= p)
        tri = resid.tile([P, 512], f16, tag="tri")
        # causal mask via PE: neye = -3e4*I, bmask[r, j] = (j < r), so
        # neye.T @ bmask adds -3e4 above the causal diagonal in PSUM
        neye = resid.tile([P, P], f16, tag="neye")
        bmask = resid.tile([P, P], f16, tag="bmask")

        def merge(*streams):
            """Weighted round-robin merge: higher weight drains faster."""
            streams = [(list(u), w) for u, w in streams if u]
            pos = [0] * len(streams)
            out = []
            while True:
                best, bf = -1, None
                for si, (u, w) in enumerate(streams):
                    if pos[si] >= len(u):
                        continue
                    f = pos[si] / (len(u) * w)
                    if bf is None or f < bf:
                        best, bf = si, f
                if best < 0:
                    return out
                out.append(streams[best][0][pos[best]])
                pos[best] += 1

        def run(units):
            for u in units:
                u()

        for _rep in range(reps):
            if True:
                # ---------- projection unit builders ----------
                def proj_units(name, w_tiles, in_dram, streampool, ppsum):
                    """One tensor's projection as a list of units."""
                    units = []
                    state = {}

                    def w_slice(d, csl):
                        return w_tiles[d // 4][:, d % 4, csl]

                    for st in range(ST_N):
                        def u_alloc(st=st):
                            state[st] = [
                                ppsum.tile([P, 512], f32, tag="pp",
                                           name=f"pp{name}{st}{_i}")
                                for _i in range(4)]
                        for eighth in range(8):
                            def u(st=st, eighth=eighth, alloc=(eighth == 0)):
                                if alloc:
                                    u_alloc(st)
                                psums = state[st]
                                x_sb = streampool.tile([P, 2, 512], f16,
                                                       tag="xs", name="x_sb")
                                for xi in range(2):
                                    nc.sync.dma_start(
                                        x_sb[:, xi],
                                        in_dram[2 * eighth + xi, st])
                                for i in range(2):
                                    d = 2 * eighth + i
                                    if name == "v":
                                        for j in range(4):
                                            nc.tensor.matmul(
                                                psums[j][:],
                                                x_sb[:, i, j * P:(j + 1) * P],
                                                w_slice(d, slice(None)),
                                                start=(d == 0),
                                                stop=(d == DC - 1))
                                    else:
                                        for c in range(4):
                                            nc.tensor.matmul(
                                                psums[c][:],
                                                w_slice(d,
                                                        slice(c * P,
                                                              (c + 1) * P)),
                                                x_sb[:, i, :],
                                                start=(d == 0),
                                                stop=(d == DC - 1))
                                if eighth == 7:
                                    for i in range(4):
                                        if name == "q":
                                            nc.vector.tensor_copy(
                                                xqT[:, i,
                                                    st * 512:(st + 1) * 512],
                                                psums[i][:])
                                        elif name == "k":
                                            nc.vector.tensor_copy(
                                                xkT[:, i,
                                                    st * 512:(st + 1) * 512],
                                                psums[i][:])
                                        else:
                                            nc.vector.tensor_copy(
                                                xv[:, 4 * st + i, :],
                                                psums[i][:])
                            units.append(u)
                    return units

                def alloc_w(pool, name, w_dram):
                    """Weight tiles + deferred per-chunk DMA closures.

                    Per-d DMAs (contiguous 128KB each) so the first matmul
                    only gates on its own chunk, not the whole tile."""
                    tiles = [None] * 4
                    dmas = []
                    for wq4 in range(4):
                        def dma(wq4=wq4):
                            wt = pool.tile([P, 4, DG], f16, tag="w",
                                           name=f"w{name}{wq4}")
                            for dd in range(4):
                                nc.sync.dma_start(wt[:, dd], w_dram[wq4, dd])
                            tiles[wq4] = wt
                        dmas.append(dma)
                    return tiles, dmas

                def weave(units, dmas, slots):
                    """Insert dma closures before the given unit indices."""
                    out = []
                    for i, u in enumerate(units):
                        while dmas and slots and i == slots[0]:
                            out.append(dmas.pop(0))
                            slots.pop(0)
                        out.append(u)
                    return out

                # ---------- attention unit builders ----------
                pending = [None]
                tstate = {}

                def norm_evict(stp, aux):
                    p_pv, p_rr, p_c, p_qt = pending[0]
                    pending[0] = None
                    # broadcast 1/denom across partitions on the (idle)
                    # GpSimd engine instead of a PE ones-matmul
                    bc_sb = aux.tile([P, 512], f16, tag="bc_sb",
                                     name="bc_sb")
                    nc.gpsimd.partition_broadcast(bc_sb[:], p_rr[:],
                                                  channels=P)
                    nc.vector.tensor_mul(
                        attnT[:, p_c, p_qt * 512:(p_qt + 1) * 512],
                        p_pv[:], bc_sb[:])

                def a_qt_units(qt, apool, aux, stp, pvp, sump, sum_tag):
                    """Software-pipelined across all 4 head-tiles of a qt
                    row: chunk j's sum/pv matmuls are emitted LAG chunks
                    later (crossing tile boundaries), so by the time they
                    reach the head of the in-order PE queue their exp input
                    is long done and the PE never stalls on ACT/DVE."""
                    units = []
                    nkc = 4 * qt + 4
                    kc_order = list(range(4 * qt, nkc)) + list(range(4 * qt))
                    chunks = [(c, ki, kc_order[ki])
                              for c in range(HPG) for ki in range(nkc)]
                    probs = {}
                    LAG = 3

                    def stage_a(c, ki, kc, qt=qt):
                        d = kc - 4 * qt
                        off = max(0, d) * P
                        w = 512 - off
                        qsl = slice(qt * 512 + off, (qt + 1) * 512)
                        st_ps = stp.tile([P, 512], f32, tag="st",
                                         name="st_ps")
                        nc.tensor.matmul(
                            st_ps[:, :w],
                            xkT[:, c, kc * P:(kc + 1) * P],
                            xqT[:, c, qsl],
                            start=True, stop=True)
                        probsT = apool.tile([P, 512], f16, tag="probsT",
                                            name="probsT")
                        nc.scalar.activation(
                            probsT[:, :w], st_ps[:, :w], Exp,
                            scale=SCALE)
                        if d >= 0:
                            mw = min(P, w)
                            nc.vector.tensor_mul(
                                probsT[:, :mw], probsT[:, :mw],
                                tri[:, :mw])
                        probs[(c, ki)] = (probsT, off, w)

                    def stage_b(c, ki, kc, qt=qt, nkc=nkc, sum_tag=sum_tag):
                        if ki == 0:
                            if pending[0] is not None:
                                norm_evict(stp, aux)
                            tstate["pv"] = pvp.tile(
                                [P, 512], f32, tag="pv", name="pv")
                            tstate["sum"] = sump.tile(
                                [1, 512], f32, tag=sum_tag, name="sum")
                        pv_ps, sum_ps = tstate["pv"], tstate["sum"]
                        probsT, off, w = probs.pop((c, ki))
                        nc.tensor.matmul(
                            sum_ps[:, off:], ones[:, 0:1],
                            probsT[:, :w],
                            start=(ki == 0), stop=(ki == nkc - 1))
                        nc.tensor.matmul(
                            pv_ps[:, off:],
                            xv[:, kc, c * P:(c + 1) * P],
                            probsT[:, :w],
                            start=(ki == 0), stop=(ki == nkc - 1))
                        if ki == nkc - 1:
                            recip = aux.tile([1, 512], f32, tag="recip",
                                             name="recip")
                            nc.vector.reciprocal_approx_fast(
                                out=recip[:], in_=sum_ps[:])
                            recip_r = aux.tile([1, 512], f16,
                                               tag="recip_r",
                                               name="recip_r")
                            nc.vector.tensor_copy(recip_r[:], recip[:])
                            pending[0] = (pv_ps, recip_r, c, qt)

                    for j, (c, ki, kc) in enumerate(chunks):
                        def u(j=j, c=c, ki=ki, kc=kc):
                            stage_a(c, ki, kc)
                            if ki == 0 and pending[0] is not None:
                                norm_evict(stp, aux)
                            if j >= LAG:
                                stage_b(*chunks[j - LAG])
                        units.append(u)

                    def utail():
                        for jj in range(len(chunks) - LAG, len(chunks)):
                            stage_b(*chunks[jj])
                    units.append(utail)
                    return units

                def o_units_for_qt(qt, wo_tiles, opool, opsum, o_tag="o"):
                    """Each (sc, dt) tile is split into two 2-matmul halves
                    so merged attention streams get finer-grained PE slack
                    for the ACT engine to keep pace."""
                    units = []
                    ostate = {}
                    for sc in range(4 * qt, 4 * qt + 4):
                        for dt in range(4):
                            def u1(sc=sc, dt=dt):
                                o_ps = opsum.tile([P, 512], f32, tag=o_tag,
                                                  name="o_ps")
                                ostate[(sc, dt)] = o_ps
                                for c in range(2):
                                    nc.tensor.matmul(
                                        o_ps[:],
                                        attnT[:, c, sc * P:(sc + 1) * P],
                                        wo_tiles[c][:,
                                                    dt * 512:(dt + 1) * 512],
                                        start=(c == 0), stop=False)
                            def u2(sc=sc, dt=dt):
                                o_ps = ostate.pop((sc, dt))
                                for c in range(2, HPG):
                                    nc.tensor.matmul(
                                        o_ps[:],
                                        attnT[:, c, sc * P:(sc + 1) * P],
                                        wo_tiles[c][:,
                                                    dt * 512:(dt + 1) * 512],
                                        start=False, stop=(c == HPG - 1))
                                o_sb = opool.tile([P, 512], f16, tag="o_sb",
                                                  name="o_sb")
                                nc.vector.tensor_copy(o_sb[:], o_ps[:])
                                nc.sync.dma_start(
                                    outp[sc * P:(sc + 1) * P,
                                         dt * 512:(dt + 1) * 512],
                                    o_sb[:])
                            units.append(u1)
                            units.append(u2)
                    return units

                # ================= schedule =================
                apool = ctx.enter_context(tc.tile_pool(name="apool", bufs=8))
                aux = ctx.enter_context(tc.tile_pool(name="aux1", bufs=2))
                wopool = ctx.enter_context(tc.tile_pool(name="wopool",
                                                        bufs=4))
                opool = ctx.enter_context(tc.tile_pool(name="opool", bufs=6))
                # Scope 1: the three projections share one weight pool
                # (wv reuses wq's freed slots) and one stream pool, with
                # weight-chunk DMAs woven between units; attention qt=0
                # interleaves into the V projection.
                with (
                    tc.tile_pool(name="wpool", bufs=8) as wpool,
                    tc.tile_pool(name="stream", bufs=8) as stream,
                    tc.tile_pool(name="ppsum", bufs=4, space="PSUM") as ppsum,
                    tc.tile_pool(name="stp1", bufs=2, space="PSUM") as stp1,
                    tc.tile_pool(name="pvp1", bufs=2, space="PSUM") as pvp1,
                ):
                    # HAM warmup: DMA-free dummy matmuls keep the PE busy
                    # through the cold window while the first weight/stream
                    # DMAs are still in flight.
                    warm_x = stream.tile([P, 512], f16, tag="xs",
                                         name="warm_x")
                    nc.gpsimd.memset(warm_x[:], 0)
                    warm_ps = stp1.tile([P, 512], f32, tag="st",
                                        name="warm_ps")
                    for _w in range(16):
                        nc.tensor.matmul(warm_ps[:], warm_x[:, 0:P],
                                         warm_x[:], start=True, stop=True)
                    wq_t, wq_d = alloc_w(wpool, "q", wq)
                    uq = proj_units("q", wq_t, qT, stream, ppsum)
                    wk_t, wk_d = alloc_w(wpool, "k", wk)
                    uk = proj_units("k", wk_t, kT, stream, ppsum)
                    wv_t, wv_d = alloc_w(wpool, "v", wv)
                    uv = proj_units("v", wv_t, vT, stream, ppsum)
                    a0 = a_qt_units(0, apool, aux, stp1, pvp1, stp1, "st")
                    # wq chunk i lands just before the unit that needs it;
                    # wk/wv chunks prefetch through the previous phase.
                    run(weave(uq, wq_d + wk_d, [0, 2, 4, 6, 12, 16, 20, 24]))
                    # consts are first needed by a0; don't let them delay
                    # the critical first weight/stream DMAs
                    nc.sync.dma_start(ones[:], onesd[:])
                    nc.sync.dma_start(tri[:], msk[:])
                    nc.sync.dma_start(neye[:], neyed[:])
                    nc.sync.dma_start(bmask[:], bmaskd[:])
                    run(weave(uk, wv_d, [12, 16, 20, 24]))
                    # prefetch wo during the V projection so scope 2 never
                    # waits on it
                    wo_tiles = []
                    for c4 in range(4):
                        wt = wopool.tile([P, DIM], f16, tag="wo",
                                         name=f"wo{c4}")
                        nc.sync.dma_start(wt[:], wo[c4])
                        wo_tiles.append(wt)
                    # A(0) needs V st=0 evictions -> gate behind uv[:8]
                    run(uv[:8])
                    run(merge((uv[8:], 1), (a0, 1)))
                    # flush (0,3)'s norm before scope-1 psum pools close
                    norm_evict(stp1, aux)
                    # bridge matmuls: keep the PE busy while the pool
                    # transition barrier drains, so HAM never re-throttles
                    bridge_ps = stp1.tile([P, 512], f32, tag="st",
                                          name="bridge_ps")
                    for _b in range(8):
                        nc.tensor.matmul(bridge_ps[:], ones[:], tri[:],
                                         start=True, stop=True)

                # Scope 2: attention qt 1..3 + output projection. Pool
                # creation order maps first-needed pools onto the banks
                # that scope 1 frees earliest.
                with (
                    tc.tile_pool(name="sump", bufs=1, space="PSUM") as sump,
                    tc.tile_pool(name="pvp", bufs=2, space="PSUM") as pvp,
                    tc.tile_pool(name="stp", bufs=3, space="PSUM") as stp,
                    tc.tile_pool(name="opsum", bufs=2, space="PSUM") as opsum,
                ):
                    for qt in range(1, ST_N):
                        au = a_qt_units(qt, apool, aux, stp, pvp,
                                        sump, "sum")
                        ou = o_units_for_qt(qt - 1, wo_tiles, opool, opsum)
                        run(merge((au, 1), (ou, 1)))
                    norm_evict(stp, aux)
                    # final o row: rotate through stp's 3 freed banks
                    run(o_units_for_qt(ST_N - 1, wo_tiles, opool, stp,
                                       o_tag="st"))
    nc.compile()
    return nc


def _get_nc(reps=1):
    key = ("nc", reps)
    if key not in _cache:
        _cache[key] = _build(reps)
    return _cache[key]


def _host_inputs(q, k, v, wq, wk, wv, wo):
    pp = np.arange(P)[:, None]
    jj = np.arange(512)[None, :]
    mask = np.ascontiguousarray((jj >= pp).astype(np.float16))
    ones = np.ones((P, P), np.float16)
    neye = np.ascontiguousarray(np.diag(
        np.full(P, -3.0e4, np.float32)).astype(np.float16))
    bmask = np.ascontiguousarray(
        (jj[:, :P] < pp).astype(np.float16))
    in_maps = []
    for core in range(NCORES):
        b, g = divmod(core, GROUPS)
        sl = slice(g * DG, (g + 1) * DG)
        def til_x(x):
            # x[b].T [din, s] -> [DC, ST_N, P, 512] contiguous blocks
            t = x[b].T.reshape(DC, P, ST_N, 512).transpose(0, 2, 1, 3)
            return np.ascontiguousarray(t, dtype=np.float16)

        def til_w(w):
            return np.ascontiguousarray(
                w[:, sl].reshape(4, 4, P, DG), dtype=np.float16)

        in_maps.append({
            "qT": til_x(q),
            "kT": til_x(k),
            "vT": til_x(v),
            "wq": til_w(wq),
            "wk": til_w(wk),
            "wv": til_w(wv),
            "wo": np.ascontiguousarray(wo[sl, :].reshape(4, P, DIM),
                                       dtype=np.float16),
            "msk": mask,
            "onesd": ones,
            "neyed": neye,
            "bmaskd": bmask,
        })
    return in_maps


def kernel(q, k, v, wq, wk, wv, wo, _trace=False, _trace_kwargs=None):
    from concourse.bass_utils import run_bass_kernel_spmd

    q = np.asarray(q, np.float32)
    k = np.asarray(k, np.float32)
    v = np.asarray(v, np.float32)
    nc = _get_nc()
    in_maps = _host_inputs(q, k, v, np.asarray(wq, np.float32),
                           np.asarray(wk, np.float32),
                           np.asarray(wv, np.float32),
                           np.asarray(wo, np.float32))
    kw = {}
    if _trace:
        kw = dict(trace=True, **(_trace_kwargs or {}))
    res = run_bass_kernel_spmd(nc, in_maps, core_ids=list(range(NCORES)), **kw)
    out = np.zeros((B, S, DIM), np.float32)
    for core in range(NCORES):
        b = core // GROUPS
        out[b] += res.results[core]["outp"].astype(np.float32)
    if _trace:
        _cache["last_results"] = res
    return out



# revision 34
# speedup vs baseline: 1.0422x; 1.0422x over previous
"""Bass/Trainium2 kernel for causal multi-head attention.

B=2, S=2048, DIM=2048, H=16 heads, HD=128.
Sharding: 8 cores = (batch b in 0..1) x (head-group g in 0..3).
Each core column-shards wq/wk/wv (4 heads = 512 cols), row-shards wo,
and produces a partial [S, DIM] output; the host sums the 4 partials
per batch (unshard of the row-sharded wo matmul).

Device-side layout trick: the host feeds q/k/v pre-transposed (qT =
q[b].T etc.), so every matmul contraction lands on the partition dim
with zero on-device transposes:
  xqT[hd, s] = wq_chunk.T @ qT_chunk          (proj, transposed out)
  xkT[hd, s] likewise; xv[s, hd] from vT as lhsT
  ST[k, q]   = xkT_chunk.T @ xqT_slice        (scores, transposed)
  softmax over k = partition dim: sum via ones-column matmul
  attnT[hd, q] = xv_chunk.T @ probsT          (PV; output pre-transposed)
  out[s, dout] = attnT_chunk.T @ wo_chunk     (final projection)
All matmul operands are float32r (TF32-like, full PE rate at N=512).
"""

import sys

sys.path.insert(0, "/opt/trn_rl_repo")

import numpy as np

B, S, DIM, H = 2, 2048, 2048, 16
HD = 128
NCORES = 8
GROUPS = 4  # head-groups (tensor parallel)
HPG = H // GROUPS  # 4 heads per group
DG = HPG * HD  # 512 = per-group projection width
P = 128
DC = DIM // P  # 16 din chunks
ST_N = S // 512  # 4 s-tiles of 512
SC_N = S // P  # 16 s-chunks of 128
SCALE = 1.0 / np.sqrt(HD)

_cache = {}


def _build(reps=1):
    from contextlib import ExitStack

    import concourse.bacc as bacc
    import concourse.mybir as mybir
    import concourse.tile as tile

    f32 = mybir.dt.float32
    f16 = mybir.dt.float16
    Exp = mybir.ActivationFunctionType.Exp

    nc = bacc.Bacc("TRN2", target_bir_lowering=False, debug=False,
                   num_devices=NCORES)

    # host-tiled layouts: every DMA reads a fully contiguous block
    qT = nc.dram_tensor("qT", [DC, ST_N, P, 512], f16, kind="ExternalInput")
    kT = nc.dram_tensor("kT", [DC, ST_N, P, 512], f16, kind="ExternalInput")
    vT = nc.dram_tensor("vT", [DC, ST_N, P, 512], f16, kind="ExternalInput")
    wq = nc.dram_tensor("wq", [4, 4, P, DG], f16, kind="ExternalInput")
    wk = nc.dram_tensor("wk", [4, 4, P, DG], f16, kind="ExternalInput")
    wv = nc.dram_tensor("wv", [4, 4, P, DG], f16, kind="ExternalInput")
    wo = nc.dram_tensor("wo", [4, P, DIM], f16, kind="ExternalInput")
    msk = nc.dram_tensor("msk", [P, 512], f16, kind="ExternalInput")
    onesd = nc.dram_tensor("onesd", [P, P], f16, kind="ExternalInput")
    neyed = nc.dram_tensor("neyed", [P, P], f16, kind="ExternalInput")
    bmaskd = nc.dram_tensor("bmaskd", [P, P], f16, kind="ExternalInput")
    outp = nc.dram_tensor("outp", [S, DIM], f16, kind="ExternalOutput")

    with tile.TileContext(nc) as tc, ExitStack() as ctx:
        # Resident for the whole kernel: projected activations + consts.
        resid = ctx.enter_context(tc.tile_pool(name="resid", bufs=1))
        _ = reps  # body below may be repeated for timing builds
        xqT = resid.tile([P, HPG, S], f16, tag="xqT")  # [hd, head, s]
        xkT = resid.tile([P, HPG, S], f16, tag="xkT")
        xv = resid.tile([P, SC_N, DG], f16, tag="xv")  # [s%128, s//128, dout]
        ones = resid.tile([P, P], f16, tag="ones")
        attnT = resid.tile([P, HPG, S], f16, tag="attnT")
        # single lower-triangle mask: tri[p, j] = (j >= p)
        tri = resid.tile([P, 512], f16, tag="tri")
        # causal mask via PE: neye = -3e4*I, bmask[r, j] = (j < r), so
        # neye.T @ bmask adds -3e4 above the causal diagonal in PSUM
        neye = resid.tile([P, P], f16, tag="neye")
        bmask = resid.tile([P, P], f16, tag="bmask")

        def merge(*streams):
            """Weighted round-robin merge: higher weight drains faster."""
            streams = [(list(u), w) for u, w in streams if u]
            pos = [0] * len(streams)
            out = []
            while True:
                best, bf = -1, None
                for si, (u, w) in enumerate(streams):
                    if pos[si] >= len(u):
                        continue
                    f = pos[si] / (len(u) * w)
                    if bf is None or f < bf:
                        best, bf = si, f
                if best < 0:
                    return out
                out.append(streams[best][0][pos[best]])
                pos[best] += 1

        def run(units):
            for u in units:
                u()

        for _rep in range(reps):
            if True:
                # ---------- projection unit builders ----------
                def proj_units(name, w_tiles, in_dram, streampool, ppsum):
                    """One tensor's projection as a list of units."""
                    units = []
                    state = {}

                    def w_slice(d, csl):
                        return w_tiles[d // 4][:, d % 4, csl]

                    for st in range(ST_N):
                        def u_alloc(st=st):
                            state[st] = [
                                ppsum.tile([P, 512], f32, tag="pp",
                                           name=f"pp{name}{st}{_i}")
                                for _i in range(4)]
                        for eighth in range(8):
                            def u(st=st, eighth=eighth, alloc=(eighth == 0)):
                                if alloc:
                                    u_alloc(st)
                                psums = state[st]
                                x_sb = streampool.tile([P, 2, 512], f16,
                                                       tag="xs", name="x_sb")
                                src = in_dram[2 * eighth:2 * eighth + 2, st]
                                nc.sync.dma_start(
                                    x_sb[:], src.rearrange("d p j -> p d j"))
                                for i in range(2):
                                    d = 2 * eighth + i
                                    if name == "v":
                                        for j in range(4):
                                            nc.tensor.matmul(
                                                psums[j][:],
                                                x_sb[:, i, j * P:(j + 1) * P],
                                                w_slice(d, slice(None)),
                                                start=(d == 0),
                                                stop=(d == DC - 1))
                                    else:
                                        for c in range(4):
                                            nc.tensor.matmul(
                                                psums[c][:],
                                                w_slice(d,
                                                        slice(c * P,
                                                              (c + 1) * P)),
                                                x_sb[:, i, :],
                                                start=(d == 0),
                                                stop=(d == DC - 1))
                                if eighth == 7:
                                    for i in range(4):
                                        if name == "q":
                                            nc.vector.tensor_copy(
                                                xqT[:, i,
                                                    st * 512:(st + 1) * 512],
                                                psums[i][:])
                                        elif name == "k":
                                            nc.vector.tensor_copy(
                                                xkT[:, i,
                                                    st * 512:(st + 1) * 512],
                                                psums[i][:])
                                        else:
                                            nc.vector.tensor_copy(
                                                xv[:, 4 * st + i, :],
                                                psums[i][:])
                            units.append(u)
                    return units

                def alloc_w(pool, name, w_dram):
                    """Weight tiles + deferred per-chunk DMA closures.

                    Per-d DMAs (contiguous 128KB each) so the first matmul
                    only gates on its own chunk, not the whole tile."""
                    tiles = [None] * 4
                    dmas = []
                    for wq4 in range(4):
                        def dma(wq4=wq4):
                            wt = pool.tile([P, 4, DG], f16, tag="w",
                                           name=f"w{name}{wq4}")
                            for dd in range(4):
                                nc.sync.dma_start(wt[:, dd], w_dram[wq4, dd])
                            tiles[wq4] = wt
                        dmas.append(dma)
                    return tiles, dmas

                def weave(units, dmas, slots):
                    """Insert dma closures before the given unit indices."""
                    out = []
                    for i, u in enumerate(units):
                        while dmas and slots and i == slots[0]:
                            out.append(dmas.pop(0))
                            slots.pop(0)
                        out.append(u)
                    return out

                # ---------- attention unit builders ----------
                pending = [None]
                tstate = {}

                def norm_evict(stp, aux):
                    p_pv, p_rr, p_c, p_qt = pending[0]
                    pending[0] = None
                    # broadcast 1/denom across partitions on the (idle)
                    # GpSimd engine instead of a PE ones-matmul
                    bc_sb = aux.tile([P, 512], f16, tag="bc_sb",
                                     name="bc_sb")
                    nc.gpsimd.partition_broadcast(bc_sb[:], p_rr[:],
                                                  channels=P)
                    nc.vector.tensor_mul(
                        attnT[:, p_c, p_qt * 512:(p_qt + 1) * 512],
                        p_pv[:], bc_sb[:])

                def a_qt_units(qt, apool, aux, stp, pvp, sump, sum_tag):
                    """Software-pipelined across all 4 head-tiles of a qt
                    row: chunk j's sum/pv matmuls are emitted LAG chunks
                    later (crossing tile boundaries), so by the time they
                    reach the head of the in-order PE queue their exp input
                    is long done and the PE never stalls on ACT/DVE."""
                    units = []
                    nkc = 4 * qt + 4
                    kc_order = list(range(4 * qt, nkc)) + list(range(4 * qt))
                    chunks = [(c, ki, kc_order[ki])
                              for c in range(HPG) for ki in range(nkc)]
                    probs = {}
                    LAG = 3

                    def stage_a(c, ki, kc, qt=qt):
                        d = kc - 4 * qt
                        off = max(0, d) * P
                        w = 512 - off
                        qsl = slice(qt * 512 + off, (qt + 1) * 512)
                        st_ps = stp.tile([P, 512], f32, tag="st",
                                         name="st_ps")
                        nc.tensor.matmul(
                            st_ps[:, :w],
                            xkT[:, c, kc * P:(kc + 1) * P],
                            xqT[:, c, qsl],
                            start=True, stop=True)
                        probsT = apool.tile([P, 512], f16, tag="probsT",
                                            name="probsT")
                        nc.scalar.activation(
                            probsT[:, :w], st_ps[:, :w], Exp,
                            scale=SCALE)
                        if d >= 0:
                            mw = min(P, w)
                            nc.vector.tensor_mul(
                                probsT[:, :mw], probsT[:, :mw],
                                tri[:, :mw])
                        probs[(c, ki)] = (probsT, off, w)

                    def stage_b(c, ki, kc, qt=qt, nkc=nkc, sum_tag=sum_tag):
                        if ki == 0:
                            if pending[0] is not None:
                                norm_evict(stp, aux)
                            tstate["pv"] = pvp.tile(
                                [P, 512], f32, tag="pv", name="pv")
                            tstate["sum"] = sump.tile(
                                [1, 512], f32, tag=sum_tag, name="sum")
                        pv_ps, sum_ps = tstate["pv"], tstate["sum"]
                        probsT, off, w = probs.pop((c, ki))
                        nc.tensor.matmul(
                            sum_ps[:, off:], ones[:, 0:1],
                            probsT[:, :w],
                            start=(ki == 0), stop=(ki == nkc - 1))
                        nc.tensor.matmul(
                            pv_ps[:, off:],
                            xv[:, kc, c * P:(c + 1) * P],
                            probsT[:, :w],
                            start=(ki == 0), stop=(ki == nkc - 1))
                        if ki == nkc - 1:
                            recip = aux.tile([1, 512], f32, tag="recip",
                                             name="recip")
                            nc.vector.reciprocal_approx_fast(
                                out=recip[:], in_=sum_ps[:])
                            recip_r = aux.tile([1, 512], f16,
                                               tag="recip_r",
                                               name="recip_r")
                            nc.vector.tensor_copy(recip_r[:], recip[:])
                            pending[0] = (pv_ps, recip_r, c, qt)

                    for j, (c, ki, kc) in enumerate(chunks):
                        def u(j=j, c=c, ki=ki, kc=kc):
                            stage_a(c, ki, kc)
                            if ki == 0 and pending[0] is not None:
                                norm_evict(stp, aux)
                            if j >= LAG:
                                stage_b(*chunks[j - LAG])
                        units.append(u)

                    def utail():
                        for jj in range(len(chunks) - LAG, len(chunks)):
                            stage_b(*chunks[jj])
                    units.append(utail)
                    return units

                def o_units_for_qt(qt, wo_tiles, opool, opsum, o_tag="o"):
                    """Each (sc, dt) tile is split into two 2-matmul halves
                    so merged attention streams get finer-grained PE slack
                    for the ACT engine to keep pace."""
                    units = []
                    ostate = {}
                    for sc in range(4 * qt, 4 * qt + 4):
                        for dt in range(4):
                            def u1(sc=sc, dt=dt):
                                o_ps = opsum.tile([P, 512], f32, tag=o_tag,
                                                  name="o_ps")
                                ostate[(sc, dt)] = o_ps
                                for c in range(2):
                                    nc.tensor.matmul(
                                        o_ps[:],
                                        attnT[:, c, sc * P:(sc + 1) * P],
                                        wo_tiles[c][:,
                                                    dt * 512:(dt + 1) * 512],
                                        start=(c == 0), stop=False)
                            def u2(sc=sc, dt=dt):
                                o_ps = ostate.pop((sc, dt))
                                for c in range(2, HPG):
                                    nc.tensor.matmul(
                                        o_ps[:],
                                        attnT[:, c, sc * P:(sc + 1) * P],
                                        wo_tiles[c][:,
                                                    dt * 512:(dt + 1) * 512],
                                        start=False, stop=(c == HPG - 1))
                                o_sb = opool.tile([P, 512], f16, tag="o_sb",
                                                  name="o_sb")
                                nc.vector.tensor_copy(o_sb[:], o_ps[:])
                                nc.sync.dma_start(
                                    outp[sc * P:(sc + 1) * P,
                                         dt * 512:(dt + 1) * 512],
                                    o_sb[:])
                            units.append(u1)
                            units.append(u2)
                    return units

                # ================= schedule =================
                apool = ctx.enter_context(tc.tile_pool(name="apool", bufs=8))
                aux = ctx.enter_context(tc.tile_pool(name="aux1", bufs=2))
                wopool = ctx.enter_context(tc.tile_pool(name="wopool",
                                                        bufs=4))
                opool = ctx.enter_context(tc.tile_pool(name="opool", bufs=6))
                # Scope 1: the three projections share one weight pool
                # (wv reuses wq's freed slots) and one stream pool, with
                # weight-chunk DMAs woven between units; attention qt=0
                # interleaves into the V projection.
                with (
                    tc.tile_pool(name="wpool", bufs=8) as wpool,
                    tc.tile_pool(name="stream", bufs=8) as stream,
                    tc.tile_pool(name="ppsum", bufs=4, space="PSUM") as ppsum,
                    tc.tile_pool(name="stp1", bufs=2, space="PSUM") as stp1,
                    tc.tile_pool(name="pvp1", bufs=2, space="PSUM") as pvp1,
                ):
                    # HAM warmup: DMA-free dummy matmuls keep the PE busy
                    # through the cold window while the first weight/stream
                    # DMAs are still in flight.
                    warm_x = stream.tile([P, 512], f16, tag="xs",
                                         name="warm_x")
                    nc.gpsimd.memset(warm_x[:], 0)
                    warm_ps = stp1.tile([P, 512], f32, tag="st",
                                        name="warm_ps")
                    for _w in range(16):
                        nc.tensor.matmul(warm_ps[:], warm_x[:, 0:P],
                                         warm_x[:], start=True, stop=True)
                    wq_t, wq_d = alloc_w(wpool, "q", wq)
                    uq = proj_units("q", wq_t, qT, stream, ppsum)
                    wk_t, wk_d = alloc_w(wpool, "k", wk)
                    uk = proj_units("k", wk_t, kT, stream, ppsum)
                    wv_t, wv_d = alloc_w(wpool, "v", wv)
                    uv = proj_units("v", wv_t, vT, stream, ppsum)
                    a0 = a_qt_units(0, apool, aux, stp1, pvp1, stp1, "st")
                    # wq chunk i lands just before the unit that needs it;
                    # wk/wv chunks prefetch through the previous phase.
                    run(weave(uq, wq_d + wk_d, [0, 2, 4, 6, 12, 16, 20, 24]))
                    # consts are first needed by a0; don't let them delay
                    # the critical first weight/stream DMAs
                    nc.sync.dma_start(ones[:], onesd[:])
                    nc.sync.dma_start(tri[:], msk[:])
                    nc.sync.dma_start(neye[:], neyed[:])
                    nc.sync.dma_start(bmask[:], bmaskd[:])
                    run(weave(uk, wv_d, [12, 16, 20, 24]))
                    # prefetch wo during the V projection so scope 2 never
                    # waits on it
                    wo_tiles = []
                    for c4 in range(4):
                        wt = wopool.tile([P, DIM], f16, tag="wo",
                                         name=f"wo{c4}")
                        nc.sync.dma_start(wt[:], wo[c4])
                        wo_tiles.append(wt)
                    # A(0) needs V st=0 evictions -> gate behind uv[:8]
                    run(uv[:8])
                    run(merge((uv[8:], 1), (a0, 1)))
                    # flush (0,3)'s norm before scope-1 psum pools close
                    norm_evict(stp1, aux)
                    # bridge matmuls: keep the PE busy while the pool
                    # transition barrier drains, so HAM never re-throttles
                    bridge_ps = stp1.tile([P, 512], f32, tag="st",
                                          name="bridge_ps")
                    for _b in range(8):
                        nc.tensor.matmul(bridge_ps[:], ones[:], tri[:],
                                         start=True, stop=True)

                # Scope 2: attention qt 1..3 + output projection. Pool
                # creation order maps first-needed pools onto the banks
                # that scope 1 frees earliest.
                with (
                    tc.tile_pool(name="sump", bufs=1, space="PSUM") as sump,
                    tc.tile_pool(name="pvp", bufs=2, space="PSUM") as pvp,
                    tc.tile_pool(name="stp", bufs=3, space="PSUM") as stp,
                    tc.tile_pool(name="opsum", bufs=2, space="PSUM") as opsum,
                ):
                    for qt in range(1, ST_N):
                        au = a_qt_units(qt, apool, aux, stp, pvp,
                                        sump, "sum")
                        ou = o_units_for_qt(qt - 1, wo_tiles, opool, opsum)
                        run(merge((au, 1), (ou, 1)))
                    norm_evict(stp, aux)
                    # final o row: rotate through stp's 3 freed banks
                    run(o_units_for_qt(ST_N - 1, wo_tiles, opool, stp,
                                       o_tag="st"))
    nc.compile()
    return nc


def _get_nc(reps=1):
    key = ("nc", reps)
    if key not in _cache:
        _cache[key] = _build(reps)
    return _cache[key]


def _host_inputs(q, k, v, wq, wk, wv, wo):
    pp = np.arange(P)[:, None]
    jj = np.arange(512)[None, :]
    mask = np.ascontiguousarray((jj >= pp).astype(np.float16))
    ones = np.ones((P, P), np.float16)
    neye = np.ascontiguousarray(np.diag(
        np.full(P, -3.0e4, np.float32)).astype(np.float16))
    bmask = np.ascontiguousarray(
        (jj[:, :P] < pp).astype(np.float16))
    in_maps = []
    for core in range(NCORES):
        b, g = divmod(core, GROUPS)
        sl = slice(g * DG, (g + 1) * DG)
        def til_x(x):
            # x[b].T [din, s] -> [DC, ST_N, P, 512] contiguous blocks
            t = x[b].T.reshape(DC, P, ST_N, 512).transpose(0, 2, 1, 3)
            return np.ascontiguousarray(t, dtype=np.float16)

        def til_w(w):
            return np.ascontiguousarray(
                w[:, sl].reshape(4, 4, P, DG), dtype=np.float16)

        in_maps.append({
            "qT": til_x(q),
            "kT": til_x(k),
            "vT": til_x(v),
            "wq": til_w(wq),
            "wk": til_w(wk),
            "wv": til_w(wv),
            "wo": np.ascontiguousarray(wo[sl, :].reshape(4, P, DIM),
                                       dtype=np.float16),
            "msk": mask,
            "onesd": ones,
            "neyed": neye,
            "bmaskd": bmask,
        })
    return in_maps


def kernel(q, k, v, wq, wk, wv, wo, _trace=False, _trace_kwargs=None):
    from concourse.bass_utils import run_bass_kernel_spmd

    q = np.asarray(q, np.float32)
    k = np.asarray(k, np.float32)
    v = np.asarray(v, np.float32)
    nc = _get_nc()
    in_maps = _host_inputs(q, k, v, np.asarray(wq, np.float32),
                           np.asarray(wk, np.float32),
                           np.asarray(wv, np.float32),
                           np.asarray(wo, np.float32))
    kw = {}
    if _trace:
        kw = dict(trace=True, **(_trace_kwargs or {}))
    res = run_bass_kernel_spmd(nc, in_maps, core_ids=list(range(NCORES)), **kw)
    out = np.zeros((B, S, DIM), np.float32)
    for core in range(NCORES):
        b = core // GROUPS
        out[b] += res.results[core]["outp"].astype(np.float32)
    if _trace:
        _cache["last_results"] = res
    return out



# revision 36
# speedup vs baseline: 1.0479x; 1.0054x over previous
"""Bass/Trainium2 kernel for causal multi-head attention.

B=2, S=2048, DIM=2048, H=16 heads, HD=128.
Sharding: 8 cores = (batch b in 0..1) x (head-group g in 0..3).
Each core column-shards wq/wk/wv (4 heads = 512 cols), row-shards wo,
and produces a partial [S, DIM] output; the host sums the 4 partials
per batch (unshard of the row-sharded wo matmul).

Device-side layout trick: the host feeds q/k/v pre-transposed (qT =
q[b].T etc.), so every matmul contraction lands on the partition dim
with zero on-device transposes:
  xqT[hd, s] = wq_chunk.T @ qT_chunk          (proj, transposed out)
  xkT[hd, s] likewise; xv[s, hd] from vT as lhsT
  ST[k, q]   = xkT_chunk.T @ xqT_slice        (scores, transposed)
  softmax over k = partition dim: sum via ones-column matmul
  attnT[hd, q] = xv_chunk.T @ probsT          (PV; output pre-transposed)
  out[s, dout] = attnT_chunk.T @ wo_chunk     (final projection)
All matmul operands are float32r (TF32-like, full PE rate at N=512).
"""

import sys

sys.path.insert(0, "/opt/trn_rl_repo")

import numpy as np

B, S, DIM, H = 2, 2048, 2048, 16
HD = 128
NCORES = 8
GROUPS = 4  # head-groups (tensor parallel)
HPG = H // GROUPS  # 4 heads per group
DG = HPG * HD  # 512 = per-group projection width
P = 128
DC = DIM // P  # 16 din chunks
ST_N = S // 512  # 4 s-tiles of 512
SC_N = S // P  # 16 s-chunks of 128
SCALE = 1.0 / np.sqrt(HD)

_cache = {}


def _build(reps=1):
    from contextlib import ExitStack

    import concourse.bacc as bacc
    import concourse.mybir as mybir
    import concourse.tile as tile

    f32 = mybir.dt.float32
    f16 = mybir.dt.float16
    Exp = mybir.ActivationFunctionType.Exp

    nc = bacc.Bacc("TRN2", target_bir_lowering=False, debug=False,
                   num_devices=NCORES)

    # host-tiled layouts: every DMA reads a fully contiguous block
    qT = nc.dram_tensor("qT", [DC, ST_N, P, 512], f16, kind="ExternalInput")
    kT = nc.dram_tensor("kT", [DC, ST_N, P, 512], f16, kind="ExternalInput")
    vT = nc.dram_tensor("vT", [DC, ST_N, P, 512], f16, kind="ExternalInput")
    wq = nc.dram_tensor("wq", [4, 4, P, DG], f16, kind="ExternalInput")
    wk = nc.dram_tensor("wk", [4, 4, P, DG], f16, kind="ExternalInput")
    wv = nc.dram_tensor("wv", [4, 4, P, DG], f16, kind="ExternalInput")
    wo = nc.dram_tensor("wo", [4, P, DIM], f16, kind="ExternalInput")
    msk = nc.dram_tensor("msk", [P, 512], f16, kind="ExternalInput")
    onesd = nc.dram_tensor("onesd", [P, P], f16, kind="ExternalInput")
    neyed = nc.dram_tensor("neyed", [P, P], f16, kind="ExternalInput")
    bmaskd = nc.dram_tensor("bmaskd", [P, P], f16, kind="ExternalInput")
    outp = nc.dram_tensor("outp", [S, DIM], f16, kind="ExternalOutput")

    with tile.TileContext(nc) as tc, ExitStack() as ctx:
        # Resident for the whole kernel: projected activations + consts.
        resid = ctx.enter_context(tc.tile_pool(name="resid", bufs=1))
        _ = reps  # body below may be repeated for timing builds
        xqT = resid.tile([P, HPG, S], f16, tag="xqT")  # [hd, head, s]
        xkT = resid.tile([P, HPG, S], f16, tag="xkT")
        xv = resid.tile([P, SC_N, DG], f16, tag="xv")  # [s%128, s//128, dout]
        ones = resid.tile([P, P], f16, tag="ones")
        attnT = resid.tile([P, HPG, S], f16, tag="attnT")
        # single lower-triangle mask: tri[p, j] = (j >= p)
        tri = resid.tile([P, 512], f16, tag="tri")
        # causal mask via PE: neye = -3e4*I, bmask[r, j] = (j < r), so
        # neye.T @ bmask adds -3e4 above the causal diagonal in PSUM
        neye = resid.tile([P, P], f16, tag="neye")
        bmask = resid.tile([P, P], f16, tag="bmask")

        def merge(*streams):
            """Weighted round-robin merge: higher weight drains faster."""
            streams = [(list(u), w) for u, w in streams if u]
            pos = [0] * len(streams)
            out = []
            while True:
                best, bf = -1, None
                for si, (u, w) in enumerate(streams):
                    if pos[si] >= len(u):
                        continue
                    f = pos[si] / (len(u) * w)
                    if bf is None or f < bf:
                        best, bf = si, f
                if best < 0:
                    return out
                out.append(streams[best][0][pos[best]])
                pos[best] += 1

        def run(units):
            for u in units:
                u()

        for _rep in range(reps):
            if True:
                # ---------- projection unit builders ----------
                def proj_units(name, w_tiles, in_dram, streampool, ppsum):
                    """One tensor's projection as a list of units."""
                    units = []
                    state = {}

                    def w_slice(d, csl):
                        return w_tiles[d // 4][:, d % 4, csl]

                    for st in range(ST_N):
                        def u_alloc(st=st):
                            state[st] = [
                                ppsum.tile([P, 512], f32, tag="pp",
                                           name=f"pp{name}{st}{_i}")
                                for _i in range(4)]
                        for eighth in range(8):
                            def u(st=st, eighth=eighth, alloc=(eighth == 0)):
                                if alloc:
                                    u_alloc(st)
                                psums = state[st]
                                x_sb = streampool.tile([P, 2, 512], f16,
                                                       tag="xs", name="x_sb")
                                src = in_dram[2 * eighth:2 * eighth + 2, st]
                                nc.sync.dma_start(
                                    x_sb[:], src.rearrange("d p j -> p d j"))
                                for i in range(2):
                                    d = 2 * eighth + i
                                    if name == "v":
                                        for j in range(4):
                                            nc.tensor.matmul(
                                                psums[j][:],
                                                x_sb[:, i, j * P:(j + 1) * P],
                                                w_slice(d, slice(None)),
                                                start=(d == 0),
                                                stop=(d == DC - 1))
                                    else:
                                        for c in range(4):
                                            nc.tensor.matmul(
                                                psums[c][:],
                                                w_slice(d,
                                                        slice(c * P,
                                                              (c + 1) * P)),
                                                x_sb[:, i, :],
                                                start=(d == 0),
                                                stop=(d == DC - 1))
                                if eighth == 7:
                                    for i in range(4):
                                        if name == "q":
                                            nc.vector.tensor_copy(
                                                xqT[:, i,
                                                    st * 512:(st + 1) * 512],
                                                psums[i][:])
                                        elif name == "k":
                                            nc.vector.tensor_copy(
                                                xkT[:, i,
                                                    st * 512:(st + 1) * 512],
                                                psums[i][:])
                                        else:
                                            nc.vector.tensor_copy(
                                                xv[:, 4 * st + i, :],
                                                psums[i][:])
                            units.append(u)
                    return units

                def alloc_w(pool, name, w_dram):
                    """Weight tiles + deferred per-chunk DMA closures.

                    Per-d DMAs (contiguous 128KB each) so the first matmul
                    only gates on its own chunk, not the whole tile."""
                    tiles = [None] * 4
                    dmas = []
                    for wq4 in range(4):
                        def dma(wq4=wq4):
                            wt = pool.tile([P, 4, DG], f16, tag="w",
                                           name=f"w{name}{wq4}")
                            for dd in range(4):
                                nc.sync.dma_start(wt[:, dd], w_dram[wq4, dd])
                            tiles[wq4] = wt
                        dmas.append(dma)
                    return tiles, dmas

                def weave(units, dmas, slots):
                    """Insert dma closures before the given unit indices."""
                    out = []
                    for i, u in enumerate(units):
                        while dmas and slots and i == slots[0]:
                            out.append(dmas.pop(0))
                            slots.pop(0)
                        out.append(u)
                    return out

                # ---------- attention unit builders ----------
                pending = [None]
                tstate = {}

                def norm_evict(stp, aux):
                    p_pv, p_rr, p_c, p_qt = pending[0]
                    pending[0] = None
                    # broadcast 1/denom across partitions on the (idle)
                    # GpSimd engine instead of a PE ones-matmul
                    bc_sb = aux.tile([P, 512], f16, tag="bc_sb",
                                     name="bc_sb")
                    nc.gpsimd.partition_broadcast(bc_sb[:], p_rr[:],
                                                  channels=P)
                    nc.vector.tensor_mul(
                        attnT[:, p_c, p_qt * 512:(p_qt + 1) * 512],
                        p_pv[:], bc_sb[:])

                def a_qt_units(qt, apool, aux, stp, pvp, sump, sum_tag):
                    """Software-pipelined across all 4 head-tiles of a qt
                    row: chunk j's sum/pv matmuls are emitted LAG chunks
                    later (crossing tile boundaries), so by the time they
                    reach the head of the in-order PE queue their exp input
                    is long done and the PE never stalls on ACT/DVE."""
                    units = []
                    nkc = 4 * qt + 4
                    kc_order = list(range(4 * qt, nkc)) + list(range(4 * qt))
                    chunks = [(c, ki, kc_order[ki])
                              for c in range(HPG) for ki in range(nkc)]
                    probs = {}
                    LAG = 3

                    def stage_a(c, ki, kc, qt=qt):
                        d = kc - 4 * qt
                        off = max(0, d) * P
                        w = 512 - off
                        qsl = slice(qt * 512 + off, (qt + 1) * 512)
                        st_ps = stp.tile([P, 512], f32, tag="st",
                                         name="st_ps")
                        nc.tensor.matmul(
                            st_ps[:, :w],
                            xkT[:, c, kc * P:(kc + 1) * P],
                            xqT[:, c, qsl],
                            start=True, stop=True)
                        probsT = apool.tile([P, 512], f16, tag="probsT",
                                            name="probsT")
                        nc.scalar.activation(
                            probsT[:, :w], st_ps[:, :w], Exp,
                            scale=SCALE)
                        if d >= 0:
                            mw = min(P, w)
                            nc.vector.tensor_mul(
                                probsT[:, :mw], probsT[:, :mw],
                                tri[:, :mw])
                        probs[(c, ki)] = (probsT, off, w)

                    def stage_b(c, ki, kc, qt=qt, nkc=nkc, sum_tag=sum_tag):
                        if ki == 0:
                            tstate["pv"] = pvp.tile(
                                [P, 512], f32, tag="pv", name="pv")
                            tstate["sum"] = sump.tile(
                                [1, 512], f32, tag=sum_tag, name="sum")
                        pv_ps, sum_ps = tstate["pv"], tstate["sum"]
                        probsT, off, w = probs.pop((c, ki))
                        nc.tensor.matmul(
                            sum_ps[:, off:], ones[:, 0:1],
                            probsT[:, :w],
                            start=(ki == 0), stop=(ki == nkc - 1))
                        nc.tensor.matmul(
                            pv_ps[:, off:],
                            xv[:, kc, c * P:(c + 1) * P],
                            probsT[:, :w],
                            start=(ki == 0), stop=(ki == nkc - 1))
                        if ki == nkc - 1:
                            recip = aux.tile([1, 512], f32, tag="recip",
                                             name="recip")
                            nc.vector.reciprocal_approx_fast(
                                out=recip[:], in_=sum_ps[:])
                            recip_r = aux.tile([1, 512], f16,
                                               tag="recip_r",
                                               name="recip_r")
                            nc.vector.tensor_copy(recip_r[:], recip[:])
                            pending[0] = (pv_ps, recip_r, c, qt)

                    del chunks
                    for c in range(HPG):
                        for ki, kc in enumerate(kc_order):
                            def u(c=c, ki=ki, kc=kc):
                                stage_a(c, ki, kc)
                                if ki == 0 and pending[0] is not None:
                                    norm_evict(stp, aux)
                                if ki >= LAG:
                                    stage_b(c, ki - LAG,
                                            kc_order[ki - LAG])
                            units.append(u)

                        def utail(c=c):
                            for kj in range(max(0, nkc - LAG), nkc):
                                stage_b(c, kj, kc_order[kj])
                        units.append(utail)
                    return units

                def o_units_for_qt(qt, wo_tiles, opool, opsum, o_tag="o"):
                    """Each (sc, dt) tile is split into two 2-matmul halves
                    so merged attention streams get finer-grained PE slack
                    for the ACT engine to keep pace."""
                    units = []
                    ostate = {}
                    for sc in range(4 * qt, 4 * qt + 4):
                        for dt in range(4):
                            def u1(sc=sc, dt=dt):
                                o_ps = opsum.tile([P, 512], f32, tag=o_tag,
                                                  name="o_ps")
                                ostate[(sc, dt)] = o_ps
                                for c in range(2):
                                    nc.tensor.matmul(
                                        o_ps[:],
                                        attnT[:, c, sc * P:(sc + 1) * P],
                                        wo_tiles[c][:,
                                                    dt * 512:(dt + 1) * 512],
                                        start=(c == 0), stop=False)
                            def u2(sc=sc, dt=dt):
                                o_ps = ostate.pop((sc, dt))
                                for c in range(2, HPG):
                                    nc.tensor.matmul(
                                        o_ps[:],
                                        attnT[:, c, sc * P:(sc + 1) * P],
                                        wo_tiles[c][:,
                                                    dt * 512:(dt + 1) * 512],
                                        start=False, stop=(c == HPG - 1))
                                o_sb = opool.tile([P, 512], f16, tag="o_sb",
                                                  name="o_sb")
                                nc.vector.tensor_copy(o_sb[:], o_ps[:])
                                nc.sync.dma_start(
                                    outp[sc * P:(sc + 1) * P,
                                         dt * 512:(dt + 1) * 512],
                                    o_sb[:])
                            units.append(u1)
                            units.append(u2)
                    return units

                # ================= schedule =================
                apool = ctx.enter_context(tc.tile_pool(name="apool", bufs=8))
                aux = ctx.enter_context(tc.tile_pool(name="aux1", bufs=2))
                wopool = ctx.enter_context(tc.tile_pool(name="wopool",
                                                        bufs=4))
                opool = ctx.enter_context(tc.tile_pool(name="opool", bufs=6))
                # Scope 1: the three projections share one weight pool
                # (wv reuses wq's freed slots) and one stream pool, with
                # weight-chunk DMAs woven between units; attention qt=0
                # interleaves into the V projection.
                with (
                    tc.tile_pool(name="wpool", bufs=8) as wpool,
                    tc.tile_pool(name="stream", bufs=8) as stream,
                    tc.tile_pool(name="ppsum", bufs=4, space="PSUM") as ppsum,
                    tc.tile_pool(name="stp1", bufs=2, space="PSUM") as stp1,
                    tc.tile_pool(name="pvp1", bufs=2, space="PSUM") as pvp1,
                ):
                    # HAM warmup: DMA-free dummy matmuls keep the PE busy
                    # through the cold window while the first weight/stream
                    # DMAs are still in flight.
                    warm_x = stream.tile([P, 512], f16, tag="xs",
                                         name="warm_x")
                    nc.gpsimd.memset(warm_x[:], 0)
                    warm_ps = stp1.tile([P, 512], f32, tag="st",
                                        name="warm_ps")
                    for _w in range(16):
                        nc.tensor.matmul(warm_ps[:], warm_x[:, 0:P],
                                         warm_x[:], start=True, stop=True)
                    wq_t, wq_d = alloc_w(wpool, "q", wq)
                    uq = proj_units("q", wq_t, qT, stream, ppsum)
                    wk_t, wk_d = alloc_w(wpool, "k", wk)
                    uk = proj_units("k", wk_t, kT, stream, ppsum)
                    wv_t, wv_d = alloc_w(wpool, "v", wv)
                    uv = proj_units("v", wv_t, vT, stream, ppsum)
                    a0 = a_qt_units(0, apool, aux, stp1, pvp1, stp1, "st")
                    # wq chunk i lands just before the unit that needs it;
                    # wk/wv chunks prefetch through the previous phase.
                    run(weave(uq, wq_d + wk_d, [0, 2, 4, 6, 12, 16, 20, 24]))
                    # consts are first needed by a0; don't let them delay
                    # the critical first weight/stream DMAs
                    nc.sync.dma_start(ones[:], onesd[:])
                    nc.sync.dma_start(tri[:], msk[:])
                    nc.sync.dma_start(neye[:], neyed[:])
                    nc.sync.dma_start(bmask[:], bmaskd[:])
                    run(weave(uk, wv_d, [12, 16, 20, 24]))
                    # prefetch wo during the V projection so scope 2 never
                    # waits on it
                    wo_tiles = []
                    for c4 in range(4):
                        wt = wopool.tile([P, DIM], f16, tag="wo",
                                         name=f"wo{c4}")
                        nc.sync.dma_start(wt[:], wo[c4])
                        wo_tiles.append(wt)
                    # A(0) needs V st=0 evictions -> gate behind uv[:8]
                    run(uv[:8])
                    run(merge((uv[8:], 1), (a0, 1)))
                    # flush (0,3)'s norm before scope-1 psum pools close
                    norm_evict(stp1, aux)
                    # bridge matmuls: keep the PE busy while the pool
                    # transition barrier drains, so HAM never re-throttles
                    bridge_ps = stp1.tile([P, 512], f32, tag="st",
                                          name="bridge_ps")
                    for _b in range(8):
                        nc.tensor.matmul(bridge_ps[:], ones[:], tri[:],
                                         start=True, stop=True)

                # Scope 2: attention qt 1..3 + output projection. Pool
                # creation order maps first-needed pools onto the banks
                # that scope 1 frees earliest.
                with (
                    tc.tile_pool(name="sump", bufs=1, space="PSUM") as sump,
                    tc.tile_pool(name="pvp", bufs=2, space="PSUM") as pvp,
                    tc.tile_pool(name="stp", bufs=3, space="PSUM") as stp,
                    tc.tile_pool(name="opsum", bufs=2, space="PSUM") as opsum,
                ):
                    for qt in range(1, ST_N):
                        au = a_qt_units(qt, apool, aux, stp, pvp,
                                        sump, "sum")
                        ou = o_units_for_qt(qt - 1, wo_tiles, opool, opsum)
                        run(merge((au, 1), (ou, 1)))
                    norm_evict(stp, aux)
                    # final o row: rotate through stp's 3 freed banks
                    run(o_units_for_qt(ST_N - 1, wo_tiles, opool, stp,
                                       o_tag="st"))
    nc.compile()
    return nc


def _get_nc(reps=1):
    key = ("nc", reps)
    if key not in _cache:
        _cache[key] = _build(reps)
    return _cache[key]


def _host_inputs(q, k, v, wq, wk, wv, wo):
    pp = np.arange(P)[:, None]
    jj = np.arange(512)[None, :]
    mask = np.ascontiguousarray((jj >= pp).astype(np.float16))
    ones = np.ones((P, P), np.float16)
    neye = np.ascontiguousarray(np.diag(
        np.full(P, -3.0e4, np.float32)).astype(np.float16))
    bmask = np.ascontiguousarray(
        (jj[:, :P] < pp).astype(np.float16))
    in_maps = []
    for core in range(NCORES):
        b, g = divmod(core, GROUPS)
        sl = slice(g * DG, (g + 1) * DG)
        def til_x(x):
            # x[b].T [din, s] -> [DC, ST_N, P, 512] contiguous blocks
            t = x[b].T.reshape(DC, P, ST_N, 512).transpose(0, 2, 1, 3)
            return np.ascontiguousarray(t, dtype=np.float16)

        def til_w(w):
            return np.ascontiguousarray(
                w[:, sl].reshape(4, 4, P, DG), dtype=np.float16)

        in_maps.append({
            "qT": til_x(q),
            "kT": til_x(k),
            "vT": til_x(v),
            "wq": til_w(wq),
            "wk": til_w(wk),
            "wv": til_w(wv),
            "wo": np.ascontiguousarray(wo[sl, :].reshape(4, P, DIM),
                                       dtype=np.float16),
            "msk": mask,
            "onesd": ones,
            "neyed": neye,
            "bmaskd": bmask,
        })
    return in_maps


def kernel(q, k, v, wq, wk, wv, wo, _trace=False, _trace_kwargs=None):
    from concourse.bass_utils import run_bass_kernel_spmd

    q = np.asarray(q, np.float32)
    k = np.asarray(k, np.float32)
    v = np.asarray(v, np.float32)
    nc = _get_nc()
    in_maps = _host_inputs(q, k, v, np.asarray(wq, np.float32),
                           np.asarray(wk, np.float32),
                           np.asarray(wv, np.float32),
                           np.asarray(wo, np.float32))
    kw = {}
    if _trace:
        kw = dict(trace=True, **(_trace_kwargs or {}))
    res = run_bass_kernel_spmd(nc, in_maps, core_ids=list(range(NCORES)), **kw)
    out = np.zeros((B, S, DIM), np.float32)
    for core in range(NCORES):
        b = core // GROUPS
        out[b] += res.results[core]["outp"].astype(np.float32)
    if _trace:
        _cache["last_results"] = res
    return out



# revision 47
# speedup vs baseline: 1.0494x; 1.0015x over previous
"""Bass/Trainium2 kernel for causal multi-head attention.

B=2, S=2048, DIM=2048, H=16 heads, HD=128.
Sharding: 8 cores = (batch b in 0..1) x (head-group g in 0..3).
Each core column-shards wq/wk/wv (4 heads = 512 cols), row-shards wo,
and produces a partial [S, DIM] output; the host sums the 4 partials
per batch (unshard of the row-sharded wo matmul).

Device-side layout trick: the host feeds q/k/v pre-transposed (qT =
q[b].T etc.), so every matmul contraction lands on the partition dim
with zero on-device transposes:
  xqT[hd, s] = wq_chunk.T @ qT_chunk          (proj, transposed out)
  xkT[hd, s] likewise; xv[s, hd] from vT as lhsT
  ST[k, q]   = xkT_chunk.T @ xqT_slice        (scores, transposed)
  softmax over k = partition dim: sum via ones-column matmul
  attnT[hd, q] = xv_chunk.T @ probsT          (PV; output pre-transposed)
  out[s, dout] = attnT_chunk.T @ wo_chunk     (final projection)
All matmul operands are float32r (TF32-like, full PE rate at N=512).
"""

import sys

sys.path.insert(0, "/opt/trn_rl_repo")

import numpy as np

B, S, DIM, H = 2, 2048, 2048, 16
HD = 128
NCORES = 8
GROUPS = 4  # head-groups (tensor parallel)
HPG = H // GROUPS  # 4 heads per group
DG = HPG * HD  # 512 = per-group projection width
P = 128
DC = DIM // P  # 16 din chunks
ST_N = S // 512  # 4 s-tiles of 512
SC_N = S // P  # 16 s-chunks of 128
SCALE = 1.0 / np.sqrt(HD)

_cache = {}


def _build(reps=1):
    from contextlib import ExitStack

    import concourse.bacc as bacc
    import concourse.mybir as mybir
    import concourse.tile as tile

    f32 = mybir.dt.float32
    f16 = mybir.dt.float16
    Exp = mybir.ActivationFunctionType.Exp

    nc = bacc.Bacc("TRN2", target_bir_lowering=False, debug=False,
                   num_devices=NCORES)

    # host-tiled layouts: every DMA reads a fully contiguous block
    qT = nc.dram_tensor("qT", [DC, ST_N, P, 512], f16, kind="ExternalInput")
    kT = nc.dram_tensor("kT", [DC, ST_N, P, 512], f16, kind="ExternalInput")
    vT = nc.dram_tensor("vT", [DC, ST_N, P, 512], f16, kind="ExternalInput")
    wq = nc.dram_tensor("wq", [4, 4, P, DG], f16, kind="ExternalInput")
    wk = nc.dram_tensor("wk", [4, 4, P, DG], f16, kind="ExternalInput")
    wv = nc.dram_tensor("wv", [4, 4, P, DG], f16, kind="ExternalInput")
    wo = nc.dram_tensor("wo", [4, P, DIM], f16, kind="ExternalInput")
    msk = nc.dram_tensor("msk", [P, 512], f16, kind="ExternalInput")
    onesd = nc.dram_tensor("onesd", [P, P], f16, kind="ExternalInput")
    neyed = nc.dram_tensor("neyed", [P, P], f16, kind="ExternalInput")
    bmaskd = nc.dram_tensor("bmaskd", [P, P], f16, kind="ExternalInput")
    outp = nc.dram_tensor("outp", [S, DIM], f16, kind="ExternalOutput")

    with tile.TileContext(nc) as tc, ExitStack() as ctx:
        # Resident for the whole kernel: projected activations + consts.
        resid = ctx.enter_context(tc.tile_pool(name="resid", bufs=1))
        _ = reps  # body below may be repeated for timing builds
        xqT = resid.tile([P, HPG, S], f16, tag="xqT")  # [hd, head, s]
        xkT = resid.tile([P, HPG, S], f16, tag="xkT")
        xv = resid.tile([P, SC_N, DG], f16, tag="xv")  # [s%128, s//128, dout]
        ones = resid.tile([P, P], f16, tag="ones")
        attnT = resid.tile([P, HPG, S], f16, tag="attnT")
        # single lower-triangle mask: tri[p, j] = (j >= p)
        tri = resid.tile([P, 512], f16, tag="tri")
        # causal mask via PE: neye = -3e4*I, bmask[r, j] = (j < r), so
        # neye.T @ bmask adds -3e4 above the causal diagonal in PSUM
        neye = resid.tile([P, P], f16, tag="neye")
        bmask = resid.tile([P, P], f16, tag="bmask")

        def merge(*streams):
            """Weighted round-robin merge: higher weight drains faster."""
            streams = [(list(u), w) for u, w in streams if u]
            pos = [0] * len(streams)
            out = []
            while True:
                best, bf = -1, None
                for si, (u, w) in enumerate(streams):
                    if pos[si] >= len(u):
                        continue
                    f = pos[si] / (len(u) * w)
                    if bf is None or f < bf:
                        best, bf = si, f
                if best < 0:
                    return out
                out.append(streams[best][0][pos[best]])
                pos[best] += 1

        def run(units):
            for u in units:
                u()

        for _rep in range(reps):
            if True:
                # ---------- projection unit builders ----------
                def proj_units(name, w_tiles, in_dram, streampool, ppsum):
                    """One tensor's projection as a list of units."""
                    units = []
                    state = {}

                    def w_slice(d, csl):
                        return w_tiles[d // 4][:, d % 4, csl]

                    for st in range(ST_N):
                        def u_alloc(st=st):
                            state[st] = [
                                ppsum.tile([P, 512], f32, tag="pp",
                                           name=f"pp{name}{st}{_i}")
                                for _i in range(4)]
                        for eighth in range(8):
                            def u(st=st, eighth=eighth, alloc=(eighth == 0)):
                                if alloc:
                                    u_alloc(st)
                                psums = state[st]
                                x_sb = streampool.tile([P, 2, 512], f16,
                                                       tag="xs", name="x_sb")
                                src = in_dram[2 * eighth:2 * eighth + 2, st]
                                nc.sync.dma_start(
                                    x_sb[:], src.rearrange("d p j -> p d j"))
                                for i in range(2):
                                    d = 2 * eighth + i
                                    if name == "v":
                                        for j in range(4):
                                            nc.tensor.matmul(
                                                psums[j][:],
                                                x_sb[:, i, j * P:(j + 1) * P],
                                                w_slice(d, slice(None)),
                                                start=(d == 0),
                                                stop=(d == DC - 1))
                                    else:
                                        for c in range(4):
                                            nc.tensor.matmul(
                                                psums[c][:],
                                                w_slice(d,
                                                        slice(c * P,
                                                              (c + 1) * P)),
                                                x_sb[:, i, :],
                                                start=(d == 0),
                                                stop=(d == DC - 1))
                                if eighth == 7:
                                    for i in range(4):
                                        if name == "q":
                                            nc.vector.tensor_copy(
                                                xqT[:, i,
                                                    st * 512:(st + 1) * 512],
                                                psums[i][:])
                                        elif name == "k":
                                            nc.vector.tensor_copy(
                                                xkT[:, i,
                                                    st * 512:(st + 1) * 512],
                                                psums[i][:])
                                        else:
                                            nc.vector.tensor_copy(
                                                xv[:, 4 * st + i, :],
                                                psums[i][:])
                            units.append(u)
                    return units

                def alloc_w(pool, name, w_dram):
                    """Weight tiles + deferred per-chunk DMA closures.

                    Per-d DMAs (contiguous 128KB each) so the first matmul
                    only gates on its own chunk, not the whole tile."""
                    tiles = [None] * 4
                    dmas = []
                    for wq4 in range(4):
                        def dma(wq4=wq4):
                            wt = pool.tile([P, 4, DG], f16, tag="w",
                                           name=f"w{name}{wq4}")
                            for dd in range(4):
                                nc.sync.dma_start(wt[:, dd], w_dram[wq4, dd])
                            tiles[wq4] = wt
                        dmas.append(dma)
                    return tiles, dmas

                def weave(units, dmas, slots):
                    """Insert dma closures before the given unit indices."""
                    out = []
                    for i, u in enumerate(units):
                        while dmas and slots and i == slots[0]:
                            out.append(dmas.pop(0))
                            slots.pop(0)
                        out.append(u)
                    return out

                # ---------- attention unit builders ----------
                pending = [None]
                tstate = {}

                def norm_evict(stp, aux, on_pe=False):
                    p_pv, p_rr, p_c, p_qt = pending[0]
                    pending[0] = None
                    if on_pe:
                        # boundary norms: PE broadcast is lower-latency and
                        # the PE is about to idle anyway
                        bc_ps = stp.tile([P, 512], f32, tag="st",
                                         name="bc_ps")
                        nc.tensor.matmul(bc_ps[:], ones[0:1, :], p_rr[:],
                                         start=True, stop=True)
                        bc = aux.tile([P, 512], f32, tag="bc_f32",
                                      name="bc_f32")
                        nc.vector.tensor_copy(bc[:], bc_ps[:])
                    else:
                        # steady state: broadcast 1/denom on the (idle)
                        # GpSimd engine instead of a PE ones-matmul
                        bc = aux.tile([P, 512], f16, tag="bc_sb",
                                      name="bc_sb")
                        nc.gpsimd.partition_broadcast(bc[:], p_rr[:],
                                                      channels=P)
                    nc.vector.tensor_mul(
                        attnT[:, p_c, p_qt * 512:(p_qt + 1) * 512],
                        p_pv[:], bc[:])

                def a_qt_units(qt, apool, aux, stp, pvp, sump, sum_tag):
                    """Software-pipelined across all 4 head-tiles of a qt
                    row: chunk j's sum/pv matmuls are emitted LAG chunks
                    later (crossing tile boundaries), so by the time they
                    reach the head of the in-order PE queue their exp input
                    is long done and the PE never stalls on ACT/DVE."""
                    units = []
                    nkc = 4 * qt + 4
                    kc_order = list(range(4 * qt, nkc)) + list(range(4 * qt))
                    chunks = [(c, ki, kc_order[ki])
                              for c in range(HPG) for ki in range(nkc)]
                    probs = {}
                    LAG = 3

                    def stage_a(c, ki, kc, qt=qt):
                        d = kc - 4 * qt
                        off = max(0, d) * P
                        w = 512 - off
                        qsl = slice(qt * 512 + off, (qt + 1) * 512)
                        st_ps = stp.tile([P, 512], f32, tag="st",
                                         name="st_ps")
                        nc.tensor.matmul(
                            st_ps[:, :w],
                            xkT[:, c, kc * P:(kc + 1) * P],
                            xqT[:, c, qsl],
                            start=True, stop=True)
                        probsT = apool.tile([P, 512], f16, tag="probsT",
                                            name="probsT")
                        nc.scalar.activation(
                            probsT[:, :w], st_ps[:, :w], Exp,
                            scale=SCALE)
                        if d >= 0:
                            mw = min(P, w)
                            nc.vector.tensor_mul(
                                probsT[:, :mw], probsT[:, :mw],
                                tri[:, :mw])
                        probs[(c, ki)] = (probsT, off, w)

                    def stage_b(c, ki, kc, qt=qt, nkc=nkc, sum_tag=sum_tag):
                        if ki == 0:
                            tstate["pv"] = pvp.tile(
                                [P, 512], f32, tag="pv", name="pv")
                            tstate["sum"] = sump.tile(
                                [1, 512], f32, tag=sum_tag, name="sum")
                        pv_ps, sum_ps = tstate["pv"], tstate["sum"]
                        probsT, off, w = probs.pop((c, ki))
                        nc.tensor.matmul(
                            sum_ps[:, off:], ones[:, 0:1],
                            probsT[:, :w],
                            start=(ki == 0), stop=(ki == nkc - 1))
                        nc.tensor.matmul(
                            pv_ps[:, off:],
                            xv[:, kc, c * P:(c + 1) * P],
                            probsT[:, :w],
                            start=(ki == 0), stop=(ki == nkc - 1))
                        if ki == nkc - 1:
                            recip = aux.tile([1, 512], f32, tag="recip",
                                             name="recip")
                            nc.vector.reciprocal_approx_fast(
                                out=recip[:], in_=sum_ps[:])
                            recip_r = aux.tile([1, 512], f16,
                                               tag="recip_r",
                                               name="recip_r")
                            nc.vector.tensor_copy(recip_r[:], recip[:])
                            pending[0] = (pv_ps, recip_r, c, qt)

                    del chunks
                    for c in range(HPG):
                        for ki, kc in enumerate(kc_order):
                            def u(c=c, ki=ki, kc=kc):
                                stage_a(c, ki, kc)
                                if ki == 0 and pending[0] is not None:
                                    norm_evict(stp, aux)
                                if ki >= LAG:
                                    stage_b(c, ki - LAG,
                                            kc_order[ki - LAG])
                            units.append(u)

                        def utail(c=c):
                            for kj in range(max(0, nkc - LAG), nkc):
                                stage_b(c, kj, kc_order[kj])
                        units.append(utail)
                    return units

                def o_units_for_qt(qt, wo_tiles, opool, opsum, o_tag="o"):
                    """Each (sc, dt) tile is split into two 2-matmul halves
                    so merged attention streams get finer-grained PE slack
                    for the ACT engine to keep pace."""
                    units = []
                    ostate = {}
                    for sc in range(4 * qt, 4 * qt + 4):
                        for dt in range(4):
                            def u1(sc=sc, dt=dt):
                                o_ps = opsum.tile([P, 512], f32, tag=o_tag,
                                                  name="o_ps")
                                ostate[(sc, dt)] = o_ps
                                for c in range(2):
                                    nc.tensor.matmul(
                                        o_ps[:],
                                        attnT[:, c, sc * P:(sc + 1) * P],
                                        wo_tiles[c][:,
                                                    dt * 512:(dt + 1) * 512],
                                        start=(c == 0), stop=False)
                            def u2(sc=sc, dt=dt):
                                o_ps = ostate.pop((sc, dt))
                                for c in range(2, HPG):
                                    nc.tensor.matmul(
                                        o_ps[:],
                                        attnT[:, c, sc * P:(sc + 1) * P],
                                        wo_tiles[c][:,
                                                    dt * 512:(dt + 1) * 512],
                                        start=False, stop=(c == HPG - 1))
                                o_sb = opool.tile([P, 512], f16, tag="o_sb",
                                                  name="o_sb")
                                nc.vector.tensor_copy(o_sb[:], o_ps[:])
                                nc.sync.dma_start(
                                    outp[sc * P:(sc + 1) * P,
                                         dt * 512:(dt + 1) * 512],
                                    o_sb[:])
                            units.append(u1)
                            units.append(u2)
                    return units

                # ================= schedule =================
                apool = ctx.enter_context(tc.tile_pool(name="apool", bufs=8))
                aux = ctx.enter_context(tc.tile_pool(name="aux1", bufs=2))
                wopool = ctx.enter_context(tc.tile_pool(name="wopool",
                                                        bufs=4))
                opool = ctx.enter_context(tc.tile_pool(name="opool", bufs=6))
                # Scope 1: the three projections share one weight pool
                # (wv reuses wq's freed slots) and one stream pool, with
                # weight-chunk DMAs woven between units; attention qt=0
                # interleaves into the V projection.
                with (
                    tc.tile_pool(name="wpool", bufs=8) as wpool,
                    tc.tile_pool(name="stream", bufs=8) as stream,
                    tc.tile_pool(name="ppsum", bufs=4, space="PSUM") as ppsum,
                    tc.tile_pool(name="stp1", bufs=2, space="PSUM") as stp1,
                    tc.tile_pool(name="pvp1", bufs=2, space="PSUM") as pvp1,
                ):
                    # HAM warmup: DMA-free dummy matmuls keep the PE busy
                    # through the cold window while the first weight/stream
                    # DMAs are still in flight.
                    warm_x = stream.tile([P, 512], f16, tag="xs",
                                         name="warm_x")
                    nc.gpsimd.memset(warm_x[:], 0)
                    warm_ps = stp1.tile([P, 512], f32, tag="st",
                                        name="warm_ps")
                    for _w in range(16):
                        nc.tensor.matmul(warm_ps[:], warm_x[:, 0:P],
                                         warm_x[:], start=True, stop=True)
                    wq_t, wq_d = alloc_w(wpool, "q", wq)
                    uq = proj_units("q", wq_t, qT, stream, ppsum)
                    wk_t, wk_d = alloc_w(wpool, "k", wk)
                    uk = proj_units("k", wk_t, kT, stream, ppsum)
                    wv_t, wv_d = alloc_w(wpool, "v", wv)
                    uv = proj_units("v", wv_t, vT, stream, ppsum)
                    a0 = a_qt_units(0, apool, aux, stp1, pvp1, stp1, "st")
                    # wq chunk i lands just before the unit that needs it;
                    # wk/wv chunks prefetch through the previous phase.
                    run(weave(uq, wq_d + wk_d, [0, 2, 4, 6, 12, 16, 20, 24]))
                    # consts are first needed by a0; don't let them delay
                    # the critical first weight/stream DMAs
                    nc.sync.dma_start(ones[:], onesd[:])
                    nc.sync.dma_start(tri[:], msk[:])
                    nc.sync.dma_start(neye[:], neyed[:])
                    nc.sync.dma_start(bmask[:], bmaskd[:])
                    run(weave(uk, wv_d, [12, 16, 20, 24]))
                    # prefetch wo during the V projection so scope 2 never
                    # waits on it
                    wo_tiles = []
                    for c4 in range(4):
                        wt = wopool.tile([P, DIM], f16, tag="wo",
                                         name=f"wo{c4}")
                        nc.sync.dma_start(wt[:], wo[c4])
                        wo_tiles.append(wt)
                    # A(0) needs V st=0 evictions -> gate behind uv[:8]
                    run(uv[:8])
                    run(merge((uv[8:], 1), (a0, 1)))
                    # flush (0,3)'s norm before scope-1 psum pools close
                    norm_evict(stp1, aux)
                    # bridge matmuls: keep the PE busy while the pool
                    # transition barrier drains, so HAM never re-throttles
                    bridge_ps = stp1.tile([P, 512], f32, tag="st",
                                          name="bridge_ps")
                    for _b in range(8):
                        nc.tensor.matmul(bridge_ps[:], ones[:], tri[:],
                                         start=True, stop=True)

                # Scope 2: attention qt 1..3 + output projection. Pool
                # creation order maps first-needed pools onto the banks
                # that scope 1 frees earliest.
                with (
                    tc.tile_pool(name="sump", bufs=1, space="PSUM") as sump,
                    tc.tile_pool(name="pvp", bufs=2, space="PSUM") as pvp,
                    tc.tile_pool(name="stp", bufs=3, space="PSUM") as stp,
                    tc.tile_pool(name="opsum", bufs=2, space="PSUM") as opsum,
                ):
                    for qt in range(1, ST_N):
                        au = a_qt_units(qt, apool, aux, stp, pvp,
                                        sump, "sum")
                        ou = o_units_for_qt(qt - 1, wo_tiles, opool, opsum)
                        run(merge((au, 1), (ou, 1)))
                    norm_evict(stp, aux)
                    # final o row: rotate through stp's 3 freed banks
                    run(o_units_for_qt(ST_N - 1, wo_tiles, opool, stp,
                                       o_tag="st"))
    nc.compile()
    return nc


def _get_nc(reps=1):
    key = ("nc", reps)
    if key not in _cache:
        _cache[key] = _build(reps)
    return _cache[key]


def _host_inputs(q, k, v, wq, wk, wv, wo):
    pp = np.arange(P)[:, None]
    jj = np.arange(512)[None, :]
    mask = np.ascontiguousarray((jj >= pp).astype(np.float16))
    ones = np.ones((P, P), np.float16)
    neye = np.ascontiguousarray(np.diag(
        np.full(P, -3.0e4, np.float32)).astype(np.float16))
    bmask = np.ascontiguousarray(
        (jj[:, :P] < pp).astype(np.float16))
    in_maps = []
    for core in range(NCORES):
        b, g = divmod(core, GROUPS)
        sl = slice(g * DG, (g + 1) * DG)
        def til_x(x):
            # x[b].T [din, s] -> [DC, ST_N, P, 512] contiguous blocks
            t = x[b].T.reshape(DC, P, ST_N, 512).transpose(0, 2, 1, 3)
            return np.ascontiguousarray(t, dtype=np.float16)

        def til_w(w):
            return np.ascontiguousarray(
                w[:, sl].reshape(4, 4, P, DG), dtype=np.float16)

        in_maps.append({
            "qT": til_x(q),
            "kT": til_x(k),
            "vT": til_x(v),
            "wq": til_w(wq),
            "wk": til_w(wk),
            "wv": til_w(wv),
            "wo": np.ascontiguousarray(wo[sl, :].reshape(4, P, DIM),
                                       dtype=np.float16),
            "msk": mask,
            "onesd": ones,
            "neyed": neye,
            "bmaskd": bmask,
        })
    return in_maps


def kernel(q, k, v, wq, wk, wv, wo, _trace=False, _trace_kwargs=None):
    from concourse.bass_utils import run_bass_kernel_spmd

    q = np.asarray(q, np.float32)
    k = np.asarray(k, np.float32)
    v = np.asarray(v, np.float32)
    nc = _get_nc()
    in_maps = _host_inputs(q, k, v, np.asarray(wq, np.float32),
                           np.asarray(wk, np.float32),
                           np.asarray(wv, np.float32),
                           np.asarray(wo, np.float32))
    kw = {}
    if _trace:
        kw = dict(trace=True, **(_trace_kwargs or {}))
    res = run_bass_kernel_spmd(nc, in_maps, core_ids=list(range(NCORES)), **kw)
    out = np.zeros((B, S, DIM), np.float32)
    for core in range(NCORES):
        b = core // GROUPS
        out[b] += res.results[core]["outp"].astype(np.float32)
    if _trace:
        _cache["last_results"] = res
    return out



# revision 48
# speedup vs baseline: 1.0506x; 1.0011x over previous
"""Bass/Trainium2 kernel for causal multi-head attention.

B=2, S=2048, DIM=2048, H=16 heads, HD=128.
Sharding: 8 cores = (batch b in 0..1) x (head-group g in 0..3).
Each core column-shards wq/wk/wv (4 heads = 512 cols), row-shards wo,
and produces a partial [S, DIM] output; the host sums the 4 partials
per batch (unshard of the row-sharded wo matmul).

Device-side layout trick: the host feeds q/k/v pre-transposed (qT =
q[b].T etc.), so every matmul contraction lands on the partition dim
with zero on-device transposes:
  xqT[hd, s] = wq_chunk.T @ qT_chunk          (proj, transposed out)
  xkT[hd, s] likewise; xv[s, hd] from vT as lhsT
  ST[k, q]   = xkT_chunk.T @ xqT_slice        (scores, transposed)
  softmax over k = partition dim: sum via ones-column matmul
  attnT[hd, q] = xv_chunk.T @ probsT          (PV; output pre-transposed)
  out[s, dout] = attnT_chunk.T @ wo_chunk     (final projection)
All matmul operands are float32r (TF32-like, full PE rate at N=512).
"""

import sys

sys.path.insert(0, "/opt/trn_rl_repo")

import numpy as np

B, S, DIM, H = 2, 2048, 2048, 16
HD = 128
NCORES = 8
GROUPS = 4  # head-groups (tensor parallel)
HPG = H // GROUPS  # 4 heads per group
DG = HPG * HD  # 512 = per-group projection width
P = 128
DC = DIM // P  # 16 din chunks
ST_N = S // 512  # 4 s-tiles of 512
SC_N = S // P  # 16 s-chunks of 128
SCALE = 1.0 / np.sqrt(HD)

_cache = {}


def _build(reps=1):
    from contextlib import ExitStack

    import concourse.bacc as bacc
    import concourse.mybir as mybir
    import concourse.tile as tile

    f32 = mybir.dt.float32
    f16 = mybir.dt.float16
    Exp = mybir.ActivationFunctionType.Exp

    nc = bacc.Bacc("TRN2", target_bir_lowering=False, debug=False,
                   num_devices=NCORES)

    # host-tiled layouts: every DMA reads a fully contiguous block
    qT = nc.dram_tensor("qT", [DC, ST_N, P, 512], f16, kind="ExternalInput")
    kT = nc.dram_tensor("kT", [DC, ST_N, P, 512], f16, kind="ExternalInput")
    vT = nc.dram_tensor("vT", [DC, ST_N, P, 512], f16, kind="ExternalInput")
    wq = nc.dram_tensor("wq", [4, 4, P, DG], f16, kind="ExternalInput")
    wk = nc.dram_tensor("wk", [4, 4, P, DG], f16, kind="ExternalInput")
    wv = nc.dram_tensor("wv", [4, 4, P, DG], f16, kind="ExternalInput")
    wo = nc.dram_tensor("wo", [4, P, DIM], f16, kind="ExternalInput")
    msk = nc.dram_tensor("msk", [P, 512], f16, kind="ExternalInput")
    onesd = nc.dram_tensor("onesd", [P, P], f16, kind="ExternalInput")
    neyed = nc.dram_tensor("neyed", [P, P], f16, kind="ExternalInput")
    bmaskd = nc.dram_tensor("bmaskd", [P, P], f16, kind="ExternalInput")
    outp = nc.dram_tensor("outp", [S, DIM], f16, kind="ExternalOutput")

    with tile.TileContext(nc) as tc, ExitStack() as ctx:
        # Resident for the whole kernel: projected activations + consts.
        resid = ctx.enter_context(tc.tile_pool(name="resid", bufs=1))
        _ = reps  # body below may be repeated for timing builds
        xqT = resid.tile([P, HPG, S], f16, tag="xqT")  # [hd, head, s]
        xkT = resid.tile([P, HPG, S], f16, tag="xkT")
        xv = resid.tile([P, SC_N, DG], f16, tag="xv")  # [s%128, s//128, dout]
        ones = resid.tile([P, P], f16, tag="ones")
        attnT = resid.tile([P, HPG, S], f16, tag="attnT")
        # single lower-triangle mask: tri[p, j] = (j >= p)
        tri = resid.tile([P, 512], f16, tag="tri")
        # causal mask via PE: neye = -3e4*I, bmask[r, j] = (j < r), so
        # neye.T @ bmask adds -3e4 above the causal diagonal in PSUM
        neye = resid.tile([P, P], f16, tag="neye")
        bmask = resid.tile([P, P], f16, tag="bmask")

        def merge(*streams):
            """Weighted round-robin merge: higher weight drains faster."""
            streams = [(list(u), w) for u, w in streams if u]
            pos = [0] * len(streams)
            out = []
            while True:
                best, bf = -1, None
                for si, (u, w) in enumerate(streams):
                    if pos[si] >= len(u):
                        continue
                    f = pos[si] / (len(u) * w)
                    if bf is None or f < bf:
                        best, bf = si, f
                if best < 0:
                    return out
                out.append(streams[best][0][pos[best]])
                pos[best] += 1

        def run(units):
            for u in units:
                u()

        for _rep in range(reps):
            if True:
                # ---------- projection unit builders ----------
                def proj_units(name, w_tiles, in_dram, streampool, ppsum):
                    """One tensor's projection as a list of units."""
                    units = []
                    state = {}

                    def w_slice(d, csl):
                        return w_tiles[d // 4][:, d % 4, csl]

                    for st in range(ST_N):
                        def u_alloc(st=st):
                            state[st] = [
                                ppsum.tile([P, 512], f32, tag="pp",
                                           name=f"pp{name}{st}{_i}")
                                for _i in range(4)]
                        for eighth in range(8):
                            def u(st=st, eighth=eighth, alloc=(eighth == 0)):
                                if alloc:
                                    u_alloc(st)
                                psums = state[st]
                                x_sb = streampool.tile([P, 2, 512], f16,
                                                       tag="xs", name="x_sb")
                                src = in_dram[2 * eighth:2 * eighth + 2, st]
                                nc.sync.dma_start(
                                    x_sb[:], src.rearrange("d p j -> p d j"))
                                for i in range(2):
                                    d = 2 * eighth + i
                                    if name == "v":
                                        for j in range(4):
                                            nc.tensor.matmul(
                                                psums[j][:],
                                                x_sb[:, i, j * P:(j + 1) * P],
                                                w_slice(d, slice(None)),
                                                start=(d == 0),
                                                stop=(d == DC - 1))
                                    else:
                                        for c in range(4):
                                            nc.tensor.matmul(
                                                psums[c][:],
                                                w_slice(d,
                                                        slice(c * P,
                                                              (c + 1) * P)),
                                                x_sb[:, i, :],
                                                start=(d == 0),
                                                stop=(d == DC - 1))
                                if eighth == 7:
                                    for i in range(4):
                                        if name == "q":
                                            nc.vector.tensor_copy(
                                                xqT[:, i,
                                                    st * 512:(st + 1) * 512],
                                                psums[i][:])
                                        elif name == "k":
                                            nc.vector.tensor_copy(
                                                xkT[:, i,
                                                    st * 512:(st + 1) * 512],
                                                psums[i][:])
                                        else:
                                            nc.vector.tensor_copy(
                                                xv[:, 4 * st + i, :],
                                                psums[i][:])
                            units.append(u)
                    return units

                def alloc_w(pool, name, w_dram, split_first=False):
                    """Weight tiles + deferred DMA closures. Only the very
                    first tile is split per-d (so the first matmul gates on
                    128KB, not 512KB); the rest issue as one DMA each to
                    keep the Sync queue's issue slots free for the
                    latency-critical x-stream DMAs."""
                    tiles = [None] * 4
                    dmas = []
                    for wq4 in range(4):
                        def dma(wq4=wq4):
                            wt = pool.tile([P, 4, DG], f16, tag="w",
                                           name=f"w{name}{wq4}")
                            if split_first and wq4 == 0:
                                for dd in range(4):
                                    nc.sync.dma_start(wt[:, dd],
                                                      w_dram[wq4, dd])
                            else:
                                nc.sync.dma_start(
                                    wt[:],
                                    w_dram[wq4].rearrange("d p n -> p d n"))
                            tiles[wq4] = wt
                        dmas.append(dma)
                    return tiles, dmas

                def weave(units, dmas, slots):
                    """Insert dma closures before the given unit indices."""
                    out = []
                    for i, u in enumerate(units):
                        while dmas and slots and i == slots[0]:
                            out.append(dmas.pop(0))
                            slots.pop(0)
                        out.append(u)
                    return out

                # ---------- attention unit builders ----------
                pending = [None]
                tstate = {}

                def norm_evict(stp, aux, on_pe=False):
                    p_pv, p_rr, p_c, p_qt = pending[0]
                    pending[0] = None
                    if on_pe:
                        # boundary norms: PE broadcast is lower-latency and
                        # the PE is about to idle anyway
                        bc_ps = stp.tile([P, 512], f32, tag="st",
                                         name="bc_ps")
                        nc.tensor.matmul(bc_ps[:], ones[0:1, :], p_rr[:],
                                         start=True, stop=True)
                        bc = aux.tile([P, 512], f32, tag="bc_f32",
                                      name="bc_f32")
                        nc.vector.tensor_copy(bc[:], bc_ps[:])
                    else:
                        # steady state: broadcast 1/denom on the (idle)
                        # GpSimd engine instead of a PE ones-matmul
                        bc = aux.tile([P, 512], f16, tag="bc_sb",
                                      name="bc_sb")
                        nc.gpsimd.partition_broadcast(bc[:], p_rr[:],
                                                      channels=P)
                    nc.vector.tensor_mul(
                        attnT[:, p_c, p_qt * 512:(p_qt + 1) * 512],
                        p_pv[:], bc[:])

                def a_qt_units(qt, apool, aux, stp, pvp, sump, sum_tag):
                    """Software-pipelined across all 4 head-tiles of a qt
                    row: chunk j's sum/pv matmuls are emitted LAG chunks
                    later (crossing tile boundaries), so by the time they
                    reach the head of the in-order PE queue their exp input
                    is long done and the PE never stalls on ACT/DVE."""
                    units = []
                    nkc = 4 * qt + 4
                    kc_order = list(range(4 * qt, nkc)) + list(range(4 * qt))
                    chunks = [(c, ki, kc_order[ki])
                              for c in range(HPG) for ki in range(nkc)]
                    probs = {}
                    LAG = 3

                    def stage_a(c, ki, kc, qt=qt):
                        d = kc - 4 * qt
                        off = max(0, d) * P
                        w = 512 - off
                        qsl = slice(qt * 512 + off, (qt + 1) * 512)
                        st_ps = stp.tile([P, 512], f32, tag="st",
                                         name="st_ps")
                        nc.tensor.matmul(
                            st_ps[:, :w],
                            xkT[:, c, kc * P:(kc + 1) * P],
                            xqT[:, c, qsl],
                            start=True, stop=True)
                        probsT = apool.tile([P, 512], f16, tag="probsT",
                                            name="probsT")
                        nc.scalar.activation(
                            probsT[:, :w], st_ps[:, :w], Exp,
                            scale=SCALE)
                        if d >= 0:
                            mw = min(P, w)
                            nc.vector.tensor_mul(
                                probsT[:, :mw], probsT[:, :mw],
                                tri[:, :mw])
                        probs[(c, ki)] = (probsT, off, w)

                    def stage_b(c, ki, kc, qt=qt, nkc=nkc, sum_tag=sum_tag):
                        if ki == 0:
                            tstate["pv"] = pvp.tile(
                                [P, 512], f32, tag="pv", name="pv")
                            tstate["sum"] = sump.tile(
                                [1, 512], f32, tag=sum_tag, name="sum")
                        pv_ps, sum_ps = tstate["pv"], tstate["sum"]
                        probsT, off, w = probs.pop((c, ki))
                        nc.tensor.matmul(
                            sum_ps[:, off:], ones[:, 0:1],
                            probsT[:, :w],
                            start=(ki == 0), stop=(ki == nkc - 1))
                        nc.tensor.matmul(
                            pv_ps[:, off:],
                            xv[:, kc, c * P:(c + 1) * P],
                            probsT[:, :w],
                            start=(ki == 0), stop=(ki == nkc - 1))
                        if ki == nkc - 1:
                            recip = aux.tile([1, 512], f32, tag="recip",
                                             name="recip")
                            nc.vector.reciprocal_approx_fast(
                                out=recip[:], in_=sum_ps[:])
                            recip_r = aux.tile([1, 512], f16,
                                               tag="recip_r",
                                               name="recip_r")
                            nc.vector.tensor_copy(recip_r[:], recip[:])
                            pending[0] = (pv_ps, recip_r, c, qt)

                    del chunks
                    for c in range(HPG):
                        for ki, kc in enumerate(kc_order):
                            def u(c=c, ki=ki, kc=kc):
                                stage_a(c, ki, kc)
                                if ki == 0 and pending[0] is not None:
                                    norm_evict(stp, aux)
                                if ki >= LAG:
                                    stage_b(c, ki - LAG,
                                            kc_order[ki - LAG])
                            units.append(u)

                        def utail(c=c):
                            for kj in range(max(0, nkc - LAG), nkc):
                                stage_b(c, kj, kc_order[kj])
                        units.append(utail)
                    return units

                def o_units_for_qt(qt, wo_tiles, opool, opsum, o_tag="o"):
                    """Each (sc, dt) tile is split into two 2-matmul halves
                    so merged attention streams get finer-grained PE slack
                    for the ACT engine to keep pace."""
                    units = []
                    ostate = {}
                    for sc in range(4 * qt, 4 * qt + 4):
                        for dt in range(4):
                            def u1(sc=sc, dt=dt):
                                o_ps = opsum.tile([P, 512], f32, tag=o_tag,
                                                  name="o_ps")
                                ostate[(sc, dt)] = o_ps
                                for c in range(2):
                                    nc.tensor.matmul(
                                        o_ps[:],
                                        attnT[:, c, sc * P:(sc + 1) * P],
                                        wo_tiles[c][:,
                                                    dt * 512:(dt + 1) * 512],
                                        start=(c == 0), stop=False)
                            def u2(sc=sc, dt=dt):
                                o_ps = ostate.pop((sc, dt))
                                for c in range(2, HPG):
                                    nc.tensor.matmul(
                                        o_ps[:],
                                        attnT[:, c, sc * P:(sc + 1) * P],
                                        wo_tiles[c][:,
                                                    dt * 512:(dt + 1) * 512],
                                        start=False, stop=(c == HPG - 1))
                                o_sb = opool.tile([P, 512], f16, tag="o_sb",
                                                  name="o_sb")
                                nc.vector.tensor_copy(o_sb[:], o_ps[:])
                                nc.sync.dma_start(
                                    outp[sc * P:(sc + 1) * P,
                                         dt * 512:(dt + 1) * 512],
                                    o_sb[:])
                            units.append(u1)
                            units.append(u2)
                    return units

                # ================= schedule =================
                apool = ctx.enter_context(tc.tile_pool(name="apool", bufs=8))
                aux = ctx.enter_context(tc.tile_pool(name="aux1", bufs=2))
                wopool = ctx.enter_context(tc.tile_pool(name="wopool",
                                                        bufs=4))
                opool = ctx.enter_context(tc.tile_pool(name="opool", bufs=6))
                # Scope 1: the three projections share one weight pool
                # (wv reuses wq's freed slots) and one stream pool, with
                # weight-chunk DMAs woven between units; attention qt=0
                # interleaves into the V projection.
                with (
                    tc.tile_pool(name="wpool", bufs=8) as wpool,
                    tc.tile_pool(name="stream", bufs=8) as stream,
                    tc.tile_pool(name="ppsum", bufs=4, space="PSUM") as ppsum,
                    tc.tile_pool(name="stp1", bufs=2, space="PSUM") as stp1,
                    tc.tile_pool(name="pvp1", bufs=2, space="PSUM") as pvp1,
                ):
                    # HAM warmup: DMA-free dummy matmuls keep the PE busy
                    # through the cold window while the first weight/stream
                    # DMAs are still in flight.
                    warm_x = stream.tile([P, 512], f16, tag="xs",
                                         name="warm_x")
                    nc.gpsimd.memset(warm_x[:], 0)
                    warm_ps = stp1.tile([P, 512], f32, tag="st",
                                        name="warm_ps")
                    for _w in range(16):
                        nc.tensor.matmul(warm_ps[:], warm_x[:, 0:P],
                                         warm_x[:], start=True, stop=True)
                    wq_t, wq_d = alloc_w(wpool, "q", wq, split_first=True)
                    uq = proj_units("q", wq_t, qT, stream, ppsum)
                    wk_t, wk_d = alloc_w(wpool, "k", wk)
                    uk = proj_units("k", wk_t, kT, stream, ppsum)
                    wv_t, wv_d = alloc_w(wpool, "v", wv)
                    uv = proj_units("v", wv_t, vT, stream, ppsum)
                    a0 = a_qt_units(0, apool, aux, stp1, pvp1, stp1, "st")
                    # wq chunk i lands just before the unit that needs it;
                    # wk/wv chunks prefetch through the previous phase.
                    run(weave(uq, wq_d + wk_d, [0, 2, 4, 6, 12, 16, 20, 24]))
                    # consts are first needed by a0; don't let them delay
                    # the critical first weight/stream DMAs
                    nc.sync.dma_start(ones[:], onesd[:])
                    nc.sync.dma_start(tri[:], msk[:])
                    nc.sync.dma_start(neye[:], neyed[:])
                    nc.sync.dma_start(bmask[:], bmaskd[:])
                    run(weave(uk, wv_d, [12, 16, 20, 24]))
                    # prefetch wo during the V projection so scope 2 never
                    # waits on it
                    wo_tiles = []
                    for c4 in range(4):
                        wt = wopool.tile([P, DIM], f16, tag="wo",
                                         name=f"wo{c4}")
                        nc.sync.dma_start(wt[:], wo[c4])
                        wo_tiles.append(wt)
                    # A(0) needs V st=0 evictions -> gate behind uv[:8]
                    run(uv[:8])
                    run(merge((uv[8:], 1), (a0, 1)))
                    # flush (0,3)'s norm before scope-1 psum pools close
                    norm_evict(stp1, aux)
                    # bridge matmuls: keep the PE busy while the pool
                    # transition barrier drains, so HAM never re-throttles
                    bridge_ps = stp1.tile([P, 512], f32, tag="st",
                                          name="bridge_ps")
                    for _b in range(8):
                        nc.tensor.matmul(bridge_ps[:], ones[:], tri[:],
                                         start=True, stop=True)

                # Scope 2: attention qt 1..3 + output projection. Pool
                # creation order maps first-needed pools onto the banks
                # that scope 1 frees earliest.
                with (
                    tc.tile_pool(name="sump", bufs=1, space="PSUM") as sump,
                    tc.tile_pool(name="pvp", bufs=2, space="PSUM") as pvp,
                    tc.tile_pool(name="stp", bufs=3, space="PSUM") as stp,
                    tc.tile_pool(name="opsum", bufs=2, space="PSUM") as opsum,
                ):
                    for qt in range(1, ST_N):
                        au = a_qt_units(qt, apool, aux, stp, pvp,
                                        sump, "sum")
                        ou = o_units_for_qt(qt - 1, wo_tiles, opool, opsum)
                        run(merge((au, 1), (ou, 1)))
                    norm_evict(stp, aux)
                    # final o row: rotate through stp's 3 freed banks
                    run(o_units_for_qt(ST_N - 1, wo_tiles, opool, stp,
                                       o_tag="st"))
    nc.compile()
    return nc


def _get_nc(reps=1):
    key = ("nc", reps)
    if key not in _cache:
        _cache[key] = _build(reps)
    return _cache[key]


def _host_inputs(q, k, v, wq, wk, wv, wo):
    pp = np.arange(P)[:, None]
    jj = np.arange(512)[None, :]
    mask = np.ascontiguousarray((jj >= pp).astype(np.float16))
    ones = np.ones((P, P), np.float16)
    neye = np.ascontiguousarray(np.diag(
        np.full(P, -3.0e4, np.float32)).astype(np.float16))
    bmask = np.ascontiguousarray(
        (jj[:, :P] < pp).astype(np.float16))
    in_maps = []
    for core in range(NCORES):
        b, g = divmod(core, GROUPS)
        sl = slice(g * DG, (g + 1) * DG)
        def til_x(x):
            # x[b].T [din, s] -> [DC, ST_N, P, 512] contiguous blocks
            t = x[b].T.reshape(DC, P, ST_N, 512).transpose(0, 2, 1, 3)
            return np.ascontiguousarray(t, dtype=np.float16)

        def til_w(w):
            return np.ascontiguousarray(
                w[:, sl].reshape(4, 4, P, DG), dtype=np.float16)

        in_maps.append({
            "qT": til_x(q),
            "kT": til_x(k),
            "vT": til_x(v),
            "wq": til_w(wq),
            "wk": til_w(wk),
            "wv": til_w(wv),
            "wo": np.ascontiguousarray(wo[sl, :].reshape(4, P, DIM),
                                       dtype=np.float16),
            "msk": mask,
            "onesd": ones,
            "neyed": neye,
            "bmaskd": bmask,
        })
    return in_maps


def kernel(q, k, v, wq, wk, wv, wo, _trace=False, _trace_kwargs=None):
    from concourse.bass_utils import run_bass_kernel_spmd

    q = np.asarray(q, np.float32)
    k = np.asarray(k, np.float32)
    v = np.asarray(v, np.float32)
    nc = _get_nc()
    in_maps = _host_inputs(q, k, v, np.asarray(wq, np.float32),
                           np.asarray(wk, np.float32),
                           np.asarray(wv, np.float32),
                           np.asarray(wo, np.float32))
    kw = {}
    if _trace:
        kw = dict(trace=True, **(_trace_kwargs or {}))
    res = run_bass_kernel_spmd(nc, in_maps, core_ids=list(range(NCORES)), **kw)
    out = np.zeros((B, S, DIM), np.float32)
    for core in range(NCORES):
        b = core // GROUPS
        out[b] += res.results[core]["outp"].astype(np.float32)
    if _trace:
        _cache["last_results"] = res
    return out

